# revision 1
# baseline (speedup 1.0000x reference)
"""Trainium2 Bass kernel for nn_Agent (conv encoder + masked LSTM scans + heads).

Sharding: data-parallel over batch B=32 across 8 cores (B_local=4). The
sequential T=64 scans run locally per core. All heavy math on device in bf16
with fp32 PSUM accumulation and fp32 cell states.
"""
import os
import sys
import numpy as np

for p in ("/opt/trn_rl_repo",):
    if p not in sys.path:
        sys.path.insert(0, p)

import ml_dtypes

bf16 = ml_dtypes.bfloat16

T, B = 64, 32
NCORES = 8
BL = B // NCORES            # 4 envs per core
TBL = T * BL                # 256 images per core
LANG_DIM, ENC_H, EMB_D, MEM_H, MEM_L, MEM_IN, NACT = 14, 256, 32, 512, 4, 288, 8

T_SCAN = int(os.environ.get("KERNEL_T_SCAN", str(T)))
CHUNK = 16                  # conv pipeline: images per chunk
NCHUNK = TBL // CHUNK

_cache = {}


def _build_nc():
    import concourse.bacc as bacc
    import concourse.tile as tile
    from concourse import mybir

    dt = mybir.dt
    AF = mybir.ActivationFunctionType
    AX = mybir.AxisListType
    F32, BF16 = dt.float32, dt.bfloat16

    nc = bacc.Bacc("TRN2", target_bir_lowering=False, debug=False,
                   enable_asserts=False, num_devices=NCORES)

    def din(name, shape, dty):
        return nc.dram_tensor(name, list(shape), dty, kind="ExternalInput")

    # ---------------- DRAM I/O ----------------
    d_im2col = din("im2col", (2, 128, TBL * 121), BF16)   # K-tiles x part x cols
    d_langm = din("langm", (16, TBL), BF16)
    d_done = din("donerow", (1, TBL), F32)
    d_oh = din("oh", (2, 128, NACT), F32)
    d_eh0 = din("eh0", (2, 128, BL), BF16)
    d_ec0 = din("ec0", (2, 128, BL), F32)
    d_mh0 = din("mh0", (MEM_L, 4, 128, BL), BF16)
    d_mc0 = din("mc0", (MEM_L, 4, 128, BL), F32)
    d_w1 = din("w1", (2, 128, 32), BF16)
    d_c1b = din("c1b", (32, 1), F32)
    d_w2 = din("w2", (3, 96, 32), BF16)
    d_c2b = din("c2b", (32, 1), F32)
    d_w3 = din("w3", (3, 96, 32), BF16)
    d_c3b = din("c3b", (32, 1), F32)
    d_fcwp = din("fcwp", (12, 128, 256), BF16)
    d_fcwl = din("fcwl", (32, 256), BF16)
    d_fcb = din("fcb", (2, 128, 1), F32)
    d_ewih = din("ewih", (16, 1024), BF16)
    d_ewhh = din("ewhh", (2, 128, 1024), BF16)
    d_ebias = din("ebias", (8, 128, 1), F32)
    d_embt = din("embt", (2, 128, 32), BF16)
    d_embb = din("embb", (32, 1), F32)
    d_w0t = din("w0t", (3, 128, 2048), BF16)
    d_wrt = din("wrt", (12, 128, 2048), BF16)
    d_wht = din("wht", (16, 128, 2048), BF16)
    d_b0 = din("b0", (16, 128, 1), F32)
    d_brep = din("brep", (3, 128, 64), F32)
    d_awct = din("awct", (4, 128, 9), BF16)
    d_awcb = din("awcb", (1, 9), BF16)
    d_out = nc.dram_tensor("out", [TBL, 3], F32, kind="ExternalOutput")

    C1 = CHUNK * 121
    C2 = CHUNK * 81
    C3 = CHUNK * 49

    with tile.TileContext(nc) as tc:
        with (
            tc.tile_pool(name="persist", bufs=1) as P,
            tc.tile_pool(name="io", bufs=2) as IO,
            tc.tile_pool(name="work", bufs=4) as W,
            tc.tile_pool(name="cpool", bufs=3) as CP,
            tc.tile_pool(name="ps", bufs=2, space="PSUM") as PS,
        ):
            # ---------- load persistent weights / tables ----------
            def ld(dram_ap, shape, dty, name):
                t = P.tile(shape, dty, tag=name)
                nc.sync.dma_start(t[:], dram_ap)
                return t

            w1 = P.tile([128, 64], BF16, tag="w1")
            for i in range(2):
                nc.sync.dma_start(w1[:, 32 * i:32 * i + 32], d_w1[i])
            c1b = ld(d_c1b[:], [32, 1], F32, "c1b")
            w2 = P.tile([96, 96], BF16, tag="w2")
            for j in range(3):
                nc.sync.dma_start(w2[:, 32 * j:32 * j + 32], d_w2[j])
            c2b = ld(d_c2b[:], [32, 1], F32, "c2b")
            w3 = P.tile([96, 96], BF16, tag="w3")
            for j in range(3):
                nc.sync.dma_start(w3[:, 32 * j:32 * j + 32], d_w3[j])
            c3b = ld(d_c3b[:], [32, 1], F32, "c3b")
            fcw = P.tile([128, 12 * 256], BF16, tag="fcw")
            for q in range(12):
                nc.sync.dma_start(fcw[:, 256 * q:256 * q + 256], d_fcwp[q])
            fcwl = ld(d_fcwl[:], [32, 256], BF16, "fcwl")
            fcb = P.tile([128, 2], F32, tag="fcb")
            for i in range(2):
                nc.sync.dma_start(fcb[:, i:i + 1], d_fcb[i])
            ewih = ld(d_ewih[:], [16, 1024], BF16, "ewih")
            ewhh = P.tile([128, 2048], BF16, tag="ewhh")
            for i in range(2):
                nc.sync.dma_start(ewhh[:, 1024 * i:1024 * i + 1024], d_ewhh[i])
            ebias = P.tile([128, 8], F32, tag="ebias")
            for i in range(8):
                nc.sync.dma_start(ebias[:, i:i + 1], d_ebias[i])
            embt = P.tile([128, 64], BF16, tag="embt")
            for i in range(2):
                nc.sync.dma_start(embt[:, 32 * i:32 * i + 32], d_embt[i])
            embb = ld(d_embb[:], [32, 1], F32, "embb")
            w0 = P.tile([128, 3 * 2048], BF16, tag="w0")
            for i in range(3):
                nc.sync.dma_start(w0[:, 2048 * i:2048 * i + 2048], d_w0t[i])
            wr = P.tile([128, 12 * 2048], BF16, tag="wr")
            for i in range(12):
                nc.sync.dma_start(wr[:, 2048 * i:2048 * i + 2048], d_wrt[i])
            wh = P.tile([128, 16 * 2048], BF16, tag="wh")
            for i in range(16):
                nc.sync.dma_start(wh[:, 2048 * i:2048 * i + 2048], d_wht[i])
            b0 = P.tile([128, 16], F32, tag="b0")
            for i in range(16):
                nc.sync.dma_start(b0[:, i:i + 1], d_b0[i])
            brep = P.tile([128, 3 * 64], F32, tag="brep")
            for i in range(3):
                nc.sync.dma_start(brep[:, 64 * i:64 * i + 64], d_brep[i])
            awct = P.tile([128, 36], BF16, tag="awct")
            for i in range(4):
                nc.sync.dma_start(awct[:, 9 * i:9 * i + 9], d_awct[i])
            awcb = ld(d_awcb[:], [1, 9], BF16, "awcb")
            ones1 = P.tile([1, 128], BF16, tag="ones1")
            nc.gpsimd.memset(ones1[:], 1.0)
            oh = P.tile([128, 16], F32, tag="oh")
            for i in range(2):
                nc.sync.dma_start(oh[:, 8 * i:8 * i + 8], d_oh[i])
            langm = ld(d_langm[:], [16, TBL], BF16, "langm")

            # done masks: (1-done) broadcast to 128 partitions via ones-matmul,
            # then replicated per hidden-tile.
            drow = P.tile([1, TBL], F32, tag="drow")
            nc.sync.dma_start(drow[:], d_done[:])
            drow_i = P.tile([1, TBL], BF16, tag="drow_i")
            nc.scalar.activation(drow_i[:], drow[:], AF.Copy, scale=-1.0, bias=1.0)
            dbc_ps = PS.tile([128, TBL], F32, tag="pre")
            nc.tensor.matmul(dbc_ps[:], ones1[:, :], drow_i[:, :], start=True, stop=True)
            dbc = P.tile([128, TBL], BF16, tag="dbc")
            nc.scalar.activation(dbc[:], dbc_ps[:], AF.Copy)
            dbc_v = dbc[:].rearrange("p (t b) -> p t b", b=BL)
            donem_l = P.tile([128, 8 * T], BF16, tag="donem_l")
            dl_v = donem_l[:].rearrange("p (t x) -> p t x", x=8)
            for j in range(2):
                nc.vector.tensor_copy(dl_v[:, :, 4 * j:4 * j + 4], dbc_v)
            donem_m = P.tile([128, 16 * T], BF16, tag="donem_m")
            dm_v = donem_m[:].rearrange("p (t x) -> p t x", x=16)
            for j in range(4):
                nc.vector.tensor_copy(dm_v[:, :, 4 * j:4 * j + 4], dbc_v)

            # persistent activations
            Zl = P.tile([128, 8 * TBL], F32, tag="Zl")
            Z0 = P.tile([128, 16 * TBL], BF16, tag="Z0")
            langH = P.tile([128, 2 * TBL], BF16, tag="langH")
            imgh = P.tile([128, 2 * TBL], BF16, tag="imgh")
            lange = P.tile([32, TBL], BF16, tag="lange")
            memH = P.tile([128, 4 * TBL], BF16, tag="memH")

            # ---------- lang LSTM: precompute input gates ----------
            for m in range(8):
                zp = PS.tile([128, TBL], F32, tag="pre")
                nc.tensor.matmul(zp[:], ewih[:, 128 * m:128 * m + 128],
                                 langm[:], start=True, stop=True)
                nc.scalar.activation(Zl[:, TBL * m:TBL * (m + 1)], zp[:],
                                     AF.Identity, bias=ebias[:, m:m + 1])

            # ---------- lang LSTM scan ----------
            eh = CP.tile([128, 8], BF16, tag="eh")
            ec = CP.tile([128, 8], F32, tag="ec")
            for i in range(2):
                nc.sync.dma_start(eh[:, 4 * i:4 * i + 4], d_eh0[i])
                nc.sync.dma_start(ec[:, 4 * i:4 * i + 4], d_ec0[i])
            Zl_v = Zl[:].rearrange("p (m n) -> p m n", n=TBL)
            for t in range(T_SCAN):
                hm = W.tile([128, 8], BF16, tag="ehm")
                cm = W.tile([128, 8], F32, tag="ecm")
                dsl = donem_l[:, 8 * t:8 * t + 8]
                nc.vector.tensor_mul(hm[:], eh[:], dsl)
                nc.vector.tensor_mul(cm[:], ec[:], dsl)
                lg = PS.tile([128, 32], F32, tag="lg")
                for m in range(8):
                    for kt in range(2):
                        nc.tensor.matmul(lg[:, 4 * m:4 * m + 4],
                                         ewhh[:, 1024 * kt + 128 * m:1024 * kt + 128 * m + 128],
                                         hm[:, 4 * kt:4 * kt + 4],
                                         start=(kt == 0), stop=(kt == 1))
                gl = W.tile([128, 32], F32, tag="gl")
                nc.vector.tensor_add(gl[:].rearrange("p (m n) -> p m n", n=4),
                                     lg[:].rearrange("p (m n) -> p m n", n=4),
                                     Zl_v[:, :, 4 * t:4 * t + 4])
                sif = W.tile([128, 16], F32, tag="esif")
                tg = W.tile([128, 8], F32, tag="etg")
                so = W.tile([128, 8], F32, tag="eso")
                nc.scalar.activation(sif[:], gl[:, 0:16], AF.Sigmoid)
                nc.scalar.activation(tg[:], gl[:, 16:24], AF.Tanh)
                nc.scalar.activation(so[:], gl[:, 24:32], AF.Sigmoid)
                t1 = W.tile([128, 8], F32, tag="et1")
                t2 = W.tile([128, 8], F32, tag="et2")
                nc.vector.tensor_mul(t1[:], sif[:, 8:16], cm[:])
                nc.vector.tensor_mul(t2[:], sif[:, 0:8], tg[:])
                ec = CP.tile([128, 8], F32, tag="ec")
                nc.vector.tensor_add(ec[:], t1[:], t2[:])
                th = W.tile([128, 8], F32, tag="eth")
                nc.scalar.activation(th[:], ec[:], AF.Tanh)
                eh = CP.tile([128, 8], BF16, tag="eh")
                nc.vector.tensor_mul(eh[:], so[:], th[:])
                nc.vector.tensor_copy(
                    langH[:].rearrange("p (j n) -> p j n", n=TBL)[:, :, BL * t:BL * t + BL],
                    eh[:].rearrange("p (j b) -> p j b", b=BL))

            # ---------- conv encoder (chunked over images) ----------
            for ch in range(NCHUNK):
                ca = IO.tile([128, C1], BF16, tag="im2a")
                cb = IO.tile([128, C1], BF16, tag="im2b")
                nc.sync.dma_start(ca[:], d_im2col[0, :, C1 * ch:C1 * (ch + 1)])
                nc.sync.dma_start(cb[:], d_im2col[1, :, C1 * ch:C1 * (ch + 1)])
                x13 = IO.tile([96, C1 + 24], BF16, tag="x13")
                for ns in range(4):  # 4 imgs per matmul: N=484
                    c1p = PS.tile([32, 484], F32, tag="cv")
                    nc.tensor.matmul(c1p[:], w1[:, 0:32],
                                     ca[:, 484 * ns:484 * (ns + 1)], start=True, stop=False)
                    nc.tensor.matmul(c1p[:], w1[:, 32:64],
                                     cb[:, 484 * ns:484 * (ns + 1)], start=False, stop=True)
                    nc.scalar.activation(x13[0:32, 484 * ns:484 * (ns + 1)], c1p[:],
                                         AF.Relu, bias=c1b[:])
                x13v = x13[0:32, 0:C1].rearrange("p (g c) -> p g c", c=121)
                for d, off in ((1, 11), (2, 22)):
                    nc.vector.tensor_copy(
                        x13[32 * d:32 * d + 32, 0:C1].rearrange(
                            "p (g c) -> p g c", c=121)[:, :, 0:121 - off],
                        x13v[:, :, off:121])
                x23 = IO.tile([96, C2 + 18], BF16, tag="x23")
                x13w = x13[:, 0:C1].rearrange("p (g a b) -> p g a b", a=11, b=11)
                for ns in range(4):  # 4 imgs per matmul: N=324
                    c2p = PS.tile([32, 4, 9, 9], F32, tag="cv")
                    for j in range(3):
                        nc.tensor.matmul(c2p[:], w2[:, 32 * j:32 * j + 32],
                                         x13w[:, 4 * ns:4 * ns + 4, 0:9, j:j + 9],
                                         start=(j == 0), stop=(j == 2))
                    nc.scalar.activation(
                        x23[0:32, 324 * ns:324 * (ns + 1)],
                        c2p[:].rearrange("p g a b -> p (g a b)"), AF.Relu, bias=c2b[:])
                x23v = x23[0:32, 0:C2].rearrange("p (g c) -> p g c", c=81)
                for d, off in ((1, 9), (2, 18)):
                    nc.vector.tensor_copy(
                        x23[32 * d:32 * d + 32, 0:C2].rearrange(
                            "p (g c) -> p g c", c=81)[:, :, 0:81 - off],
                        x23v[:, :, off:81])
                x34 = IO.tile([128, C3 + 3], BF16, tag="x34")
                x23w = x23[:, 0:C2].rearrange("p (g a b) -> p g a b", a=9, b=9)
                for ns in range(4):  # 4 imgs per matmul: N=196
                    c3p = PS.tile([32, 4, 7, 7], F32, tag="cv")
                    for j in range(3):
                        nc.tensor.matmul(c3p[:], w3[:, 32 * j:32 * j + 32],
                                         x23w[:, 4 * ns:4 * ns + 4, 0:7, j:j + 7],
                                         start=(j == 0), stop=(j == 2))
                    nc.scalar.activation(
                        x34[0:32, 196 * ns:196 * (ns + 1)],
                        c3p[:].rearrange("p g a b -> p (g a b)"), AF.Relu, bias=c3b[:])
                x34v = x34[0:32, 0:C3].rearrange("p (g c) -> p g c", c=49)
                for d in (1, 2, 3):
                    nc.vector.tensor_copy(
                        x34[32 * d:32 * d + 32, 0:C3].rearrange(
                            "p (g c) -> p g c", c=49)[:, :, 0:49 - d],
                        x34v[:, :, d:49])
                x34w = x34[:, 0:C3].rearrange("p (g c) -> p g c", c=49)
                for mt in range(2):
                    fp = PS.tile([128, CHUNK], F32, tag="cv")
                    for q in range(12):
                        nc.tensor.matmul(fp[:],
                                         fcw[:, 256 * q + 128 * mt:256 * q + 128 * mt + 128],
                                         x34w[:, :, 4 * q:4 * q + 1].opt(),
                                         start=(q == 0), stop=False)
                    nc.tensor.matmul(fp[:], fcwl[:, 128 * mt:128 * mt + 128],
                                     x34w[0:32, :, 48:49].opt(), start=False, stop=True)
                    nc.scalar.activation(
                        imgh[:, TBL * mt + CHUNK * ch:TBL * mt + CHUNK * (ch + 1)],
                        fp[:], AF.Relu, bias=fcb[:, mt:mt + 1])

            # ---------- lang embedding ----------
            ep = PS.tile([32, TBL], F32, tag="pre")
            for kt in range(2):
                nc.tensor.matmul(ep[:], embt[:, 32 * kt:32 * kt + 32],
                                 langH[:, TBL * kt:TBL * (kt + 1)],
                                 start=(kt == 0), stop=(kt == 1))
            nc.scalar.activation(lange[:], ep[:], AF.Relu, bias=embb[:])

            # ---------- mem LSTM: precompute layer-0 input gates ----------
            for m in range(16):
                z0p = PS.tile([128, TBL], F32, tag="pre")
                nc.tensor.matmul(z0p[:], w0[:, 128 * m:128 * m + 128],
                                 imgh[:, 0:TBL], start=True, stop=False)
                nc.tensor.matmul(z0p[:], w0[:, 2048 + 128 * m:2048 + 128 * m + 128],
                                 imgh[:, TBL:2 * TBL], start=False, stop=False)
                nc.tensor.matmul(z0p[:], w0[0:32, 4096 + 128 * m:4096 + 128 * m + 128],
                                 lange[:], start=False, stop=True)
                nc.scalar.activation(Z0[:, TBL * m:TBL * (m + 1)], z0p[:],
                                     AF.Identity, bias=b0[:, m:m + 1])

            # ---------- mem LSTM scan ----------
            mh = []
            mc = []
            for l in range(MEM_L):
                h_ = CP.tile([128, 16], BF16, tag=f"mh{l}")
                c_ = CP.tile([128, 16], F32, tag=f"mc{l}")
                for i in range(4):
                    nc.sync.dma_start(h_[:, 4 * i:4 * i + 4], d_mh0[l, i])
                    nc.sync.dma_start(c_[:, 4 * i:4 * i + 4], d_mc0[l, i])
                mh.append(h_)
                mc.append(c_)
            Z0_v = Z0[:].rearrange("p (m n) -> p m n", n=TBL)
            brep_v = brep[:].rearrange("p (l m n) -> p l m n", l=3, n=4)
            for t in range(T_SCAN):
                dsl = donem_m[:, 16 * t:16 * t + 16]
                xin = None
                for l in range(MEM_L):
                    hmm = W.tile([128, 16], BF16, tag=f"hm{l}")
                    cmm = W.tile([128, 16], F32, tag=f"cm{l}")
                    nc.vector.tensor_mul(hmm[:], mh[l][:], dsl)
                    nc.vector.tensor_mul(cmm[:], mc[l][:], dsl)
                    gp = PS.tile([128, 64], F32, tag="gp")
                    for m in range(16):
                        if l > 0:
                            base = ((l - 1) * 4) * 2048
                            for kt in range(4):
                                nc.tensor.matmul(
                                    gp[:, 4 * m:4 * m + 4],
                                    wr[:, base + kt * 2048 + 128 * m:base + kt * 2048 + 128 * m + 128],
                                    xin[:, 4 * kt:4 * kt + 4],
                                    start=(kt == 0), stop=False)
                        base = (l * 4) * 2048
                        for kt in range(4):
                            nc.tensor.matmul(
                                gp[:, 4 * m:4 * m + 4],
                                wh[:, base + kt * 2048 + 128 * m:base + kt * 2048 + 128 * m + 128],
                                hmm[:, 4 * kt:4 * kt + 4],
                                start=(l == 0 and kt == 0), stop=(kt == 3))
                    gs = W.tile([128, 64], F32, tag="gs")
                    if l == 0:
                        nc.vector.tensor_add(gs[:].rearrange("p (m n) -> p m n", n=4),
                                             gp[:].rearrange("p (m n) -> p m n", n=4),
                                             Z0_v[:, :, 4 * t:4 * t + 4])
                    else:
                        nc.vector.tensor_add(gs[:].rearrange("p (m n) -> p m n", n=4),
                                             gp[:].rearrange("p (m n) -> p m n", n=4),
                                             brep_v[:, l - 1])
                    sif = W.tile([128, 32], F32, tag="msif")
                    tg = W.tile([128, 16], F32, tag="mtg")
                    so = W.tile([128, 16], F32, tag="mso")
                    nc.scalar.activation(sif[:], gs[:, 0:32], AF.Sigmoid)
                    nc.scalar.activation(tg[:], gs[:, 32:48], AF.Tanh)
                    nc.scalar.activation(so[:], gs[:, 48:64], AF.Sigmoid)
                    t1 = W.tile([128, 16], F32, tag=f"mt1{l}")
                    t2 = W.tile([128, 16], F32, tag=f"mt2{l}")
                    nc.vector.tensor_mul(t1[:], sif[:, 16:32], cmm[:])
                    nc.vector.tensor_mul(t2[:], sif[:, 0:16], tg[:])
                    c_ = CP.tile([128, 16], F32, tag=f"mc{l}")
                    nc.vector.tensor_add(c_[:], t1[:], t2[:])
                    mc[l] = c_
                    th = W.tile([128, 16], F32, tag=f"mth{l}")
                    nc.scalar.activation(th[:], c_[:], AF.Tanh)
                    h_ = CP.tile([128, 16], BF16, tag=f"mh{l}")
                    nc.vector.tensor_mul(h_[:], so[:], th[:])
                    mh[l] = h_
                    xin = h_
                nc.vector.tensor_copy(
                    memH[:].rearrange("p (j n) -> p j n", n=TBL)[:, :, BL * t:BL * t + BL],
                    mh[3][:].rearrange("p (j b) -> p j b", b=BL))

            # ---------- heads ----------
            for it in range(2):
                hd = PS.tile([128, 9], F32, tag="cv")
                for kt in range(4):
                    nc.tensor.matmul(hd[:],
                                     memH[:, TBL * kt + 128 * it:TBL * kt + 128 * it + 128],
                                     awct[:, 9 * kt:9 * kt + 9],
                                     start=(kt == 0), stop=False)
                nc.tensor.matmul(hd[:], ones1[:, :], awcb[:, :], start=False, stop=True)
                mx = W.tile([128, 1], F32, tag="hmx")
                nc.vector.reduce_max(mx[:], hd[:, 0:8], axis=AX.X)
                xm = W.tile([128, 8], F32, tag="hxm")
                nc.vector.tensor_scalar_sub(xm[:], hd[:, 0:8], mx[:])
                ex = W.tile([128, 8], F32, tag="hex")
                se = W.tile([128, 1], F32, tag="hse")
                nc.scalar.activation(ex[:], xm[:], AF.Exp, accum_out=se[:])
                lnv = W.tile([128, 1], F32, tag="hln")
                nc.scalar.activation(lnv[:], se[:], AF.Ln)
                logp = W.tile([128, 8], F32, tag="hlp")
                nc.vector.tensor_scalar_sub(logp[:], xm[:], lnv[:])
                lp1 = W.tile([128, 8], F32, tag="hlp1")
                nc.vector.tensor_mul(lp1[:], logp[:], oh[:, 8 * it:8 * it + 8])
                lpa = W.tile([128, 1], F32, tag="hlpa")
                nc.vector.reduce_sum(lpa[:], lp1[:], axis=AX.X)
                t3 = W.tile([128, 8], F32, tag="ht3")
                nc.vector.tensor_mul(t3[:], ex[:], xm[:])
                sxm = W.tile([128, 1], F32, tag="hsxm")
                nc.vector.reduce_sum(sxm[:], t3[:], axis=AX.X)
                rse = W.tile([128, 1], F32, tag="hrse")
                nc.vector.reciprocal(rse[:], se[:])
                m1 = W.tile([128, 1], F32, tag="hm1")
                nc.vector.tensor_mul(m1[:], sxm[:], rse[:])
                ent = W.tile([128, 1], F32, tag="hent")
                nc.vector.tensor_sub(ent[:], lnv[:], m1[:])
                osb = W.tile([128, 3], F32, tag="osb")
                nc.vector.tensor_copy(osb[:, 0:1], lpa[:])
                nc.vector.tensor_copy(osb[:, 1:2], ent[:])
                nc.vector.tensor_copy(osb[:, 2:3], hd[:, 8:9])
                nc.sync.dma_start(d_out[128 * it:128 * (it + 1), :], osb[:])

    nc.compile()
    return nc


def _prep_shared(inputs):
    f32 = np.float32
    out = {}
    w1p = np.concatenate([inputs["conv1_w"].reshape(16, 243).T.astype(f32),
                          np.zeros((13, 16), f32)], 0)          # (256,16)
    out["w1"] = np.concatenate([w1p, w1p], 1).reshape(2, 128, 32).astype(bf16)
    out["c1b"] = np.concatenate([inputs["conv1_b"]] * 2).reshape(32, 1).astype(f32)
    w2z = np.zeros((3, 96, 32), f32)
    for j in range(3):
        for d in range(3):
            w2z[j, 32 * d:32 * d + 16, :] = inputs["conv2_w"][:, :, d, j].T
    out["w2"] = w2z.astype(bf16)
    out["c2b"] = inputs["conv2_b"].reshape(32, 1).astype(f32)
    w3 = np.stack([inputs["conv3_w"][:, :, d, :] for d in range(3)])  # (3,32,32,3)
    out["w3"] = np.ascontiguousarray(
        w3.transpose(3, 0, 2, 1).reshape(3, 96, 32)).astype(bf16)
    out["c3b"] = inputs["conv3_b"].reshape(32, 1).astype(f32)
    F = inputs["fc_w"].reshape(256, 32, 49)
    fcwp = np.stack([
        np.ascontiguousarray(F[:, :, 4 * q:4 * q + 4].transpose(2, 1, 0)).reshape(128, 256)
        for q in range(12)])
    out["fcwp"] = fcwp.astype(bf16)
    out["fcwl"] = np.ascontiguousarray(F[:, :, 48].T).astype(bf16)
    out["fcb"] = inputs["fc_b"].reshape(2, 128, 1).astype(f32)
    out["ewih"] = np.concatenate(
        [inputs["enc_Wih"].T.astype(f32), np.zeros((2, 1024), f32)], 0).astype(bf16)
    out["ewhh"] = np.ascontiguousarray(
        inputs["enc_Whh"].T.reshape(2, 128, 1024)).astype(bf16)
    out["ebias"] = (inputs["enc_bih"] + inputs["enc_bhh"]).reshape(8, 128, 1).astype(f32)
    out["embt"] = np.ascontiguousarray(
        inputs["emb_w"].T.reshape(2, 128, 32)).astype(bf16)
    out["embb"] = inputs["emb_b"].reshape(32, 1).astype(f32)
    w0 = np.concatenate([inputs["mem_Wih0"].T.astype(f32),
                         np.zeros((96, 2048), f32)], 0)  # pad 288->384
    out["w0t"] = w0.reshape(3, 128, 2048).astype(bf16)
    out["wrt"] = np.ascontiguousarray(
        inputs["mem_WihR"].transpose(0, 2, 1).reshape(12, 128, 2048)).astype(bf16)
    out["wht"] = np.ascontiguousarray(
        inputs["mem_Whh"].transpose(0, 2, 1).reshape(16, 128, 2048)).astype(bf16)
    bias = (inputs["mem_bih"] + inputs["mem_bhh"]).astype(f32)  # (4, 2048)
    out["b0"] = np.ascontiguousarray(bias[0].reshape(16, 128, 1))
    out["brep"] = np.ascontiguousarray(np.repeat(
        bias[1:].reshape(3, 16, 128).transpose(0, 2, 1), BL, axis=2))  # (3,128,64)
    out["awct"] = np.ascontiguousarray(np.concatenate(
        [inputs["actor_w"], inputs["critic_w"]], 0).T.reshape(4, 128, 9)).astype(bf16)
    out["awcb"] = np.concatenate(
        [inputs["actor_b"], inputs["critic_b"]]).reshape(1, 9).astype(bf16)
    return out


def _prep_core(inputs, k):
    f32 = np.float32
    out = {}
    img = np.asarray(inputs["img"], f32).reshape(T, B, 3, 11, 9, 11, 9)
    imk = img[:, BL * k:BL * (k + 1)]                      # (64,4,3,11,9,11,9)
    im2 = np.ascontiguousarray(
        imk.transpose(2, 4, 6, 0, 1, 3, 5)).reshape(243, TBL * 121)
    im2 = np.concatenate([im2, np.zeros((13, TBL * 121), f32)], 0)
    out["im2col"] = im2.reshape(2, 128, TBL * 121).astype(bf16)
    lk = np.asarray(inputs["lang"], f32)[:, BL * k:BL * (k + 1)]   # (64,4,14)
    lm = np.ascontiguousarray(lk.transpose(2, 0, 1)).reshape(14, TBL)
    out["langm"] = np.concatenate([lm, np.zeros((2, TBL), f32)], 0).astype(bf16)
    out["donerow"] = np.ascontiguousarray(
        np.asarray(inputs["done"], f32)[:, BL * k:BL * (k + 1)]).reshape(1, TBL)
    act = np.asarray(inputs["action"]).reshape(T, B)[:, BL * k:BL * (k + 1)].reshape(TBL)
    ohm = np.zeros((TBL, NACT), f32)
    ohm[np.arange(TBL), act.astype(np.int64)] = 1.0
    out["oh"] = ohm.reshape(2, 128, NACT)
    eh0 = np.ascontiguousarray(
        np.asarray(inputs["enc_h0"], f32)[BL * k:BL * (k + 1)].T)   # (256,4)
    ec0 = np.ascontiguousarray(
        np.asarray(inputs["enc_c0"], f32)[BL * k:BL * (k + 1)].T)
    out["eh0"] = eh0.reshape(2, 128, BL).astype(bf16)
    out["ec0"] = ec0.reshape(2, 128, BL)
    mh0 = np.ascontiguousarray(
        np.asarray(inputs["mem_h0"], f32)[:, BL * k:BL * (k + 1)].transpose(0, 2, 1))
    mc0 = np.ascontiguousarray(
        np.asarray(inputs["mem_c0"], f32)[:, BL * k:BL * (k + 1)].transpose(0, 2, 1))
    out["mh0"] = mh0.reshape(MEM_L, 4, 128, BL).astype(bf16)
    out["mc0"] = mc0.reshape(MEM_L, 4, 128, BL)
    return out


def kernel(**inputs):
    from concourse import bass_utils

    if "nc" not in _cache:
        _cache["nc"] = _build_nc()
    nc = _cache["nc"]

    shared = _prep_shared({k: np.asarray(v) for k, v in inputs.items()
                           if k not in ("img", "lang", "done", "action",
                                        "enc_h0", "enc_c0", "mem_h0", "mem_c0")})
    in_maps = []
    for k in range(NCORES):
        m = dict(shared)
        m.update(_prep_core(inputs, k))
        in_maps.append(m)

    res = bass_utils.run_bass_kernel_spmd(nc, in_maps, core_ids=list(range(NCORES)),
                                          trace=bool(int(os.environ.get("KERNEL_TRACE", "0"))))
    out_full = np.zeros((T, B, 3), np.float32)
    for k in range(NCORES):
        out_full[:, BL * k:BL * (k + 1)] = res.results[k]["out"].reshape(T, BL, 3)
    if os.environ.get("KERNEL_RESULT_STASH"):
        _cache["last_res"] = res
    return out_full.reshape(T * B, 3)



# revision 4
# speedup vs baseline: 1.0108x; 1.0108x over previous
"""Trainium2 Bass kernel for nn_Agent (conv encoder + masked LSTM scans + heads).

Sharding: data-parallel over batch B=32 across 8 cores (B_local=4). The
sequential T=64 scans run locally per core. All heavy math on device in bf16
with fp32 PSUM accumulation and fp32 cell states.
"""
import os
import sys
import numpy as np

for p in ("/opt/trn_rl_repo",):
    if p not in sys.path:
        sys.path.insert(0, p)

import ml_dtypes

bf16 = ml_dtypes.bfloat16
f8 = ml_dtypes.float8_e4m3
FP8_SCALE = 1024.0          # keep fp8-stored weights out of denormal range
INV_SCALE = 1.0 / FP8_SCALE

T, B = 64, 32
NCORES = 8
BL = B // NCORES            # 4 envs per core
TBL = T * BL                # 256 images per core
LANG_DIM, ENC_H, EMB_D, MEM_H, MEM_L, MEM_IN, NACT = 14, 256, 32, 512, 4, 288, 8

T_SCAN = int(os.environ.get("KERNEL_T_SCAN", str(T)))
CHUNK = 16                  # conv pipeline: images per chunk
NCHUNK = TBL // CHUNK

_cache = {}


def _build_nc():
    import concourse.bacc as bacc
    import concourse.tile as tile
    from concourse import mybir

    dt = mybir.dt
    AF = mybir.ActivationFunctionType
    AX = mybir.AxisListType
    F32, BF16, F8 = dt.float32, dt.bfloat16, dt.float8e4

    nc = bacc.Bacc("TRN2", target_bir_lowering=False, debug=False,
                   enable_asserts=False, num_devices=NCORES)

    def din(name, shape, dty):
        return nc.dram_tensor(name, list(shape), dty, kind="ExternalInput")

    # ---------------- DRAM I/O ----------------
    d_im2col = din("im2col", (2, 128, TBL * 121), BF16)   # K-tiles x part x cols
    d_langm = din("langm", (16, TBL), BF16)
    d_done = din("donerow", (1, TBL), F32)
    d_oh = din("oh", (2, 128, NACT), F32)
    d_eh0 = din("eh0", (2, 128, BL), BF16)
    d_ec0 = din("ec0", (2, 128, BL), F32)
    d_mh0 = din("mh0", (MEM_L, 4, 128, BL), BF16)
    d_mc0 = din("mc0", (MEM_L, 4, 128, BL), F32)
    d_w1 = din("w1", (2, 128, 32), BF16)
    d_c1b = din("c1b", (32, 1), F32)
    d_w2 = din("w2", (3, 96, 32), BF16)
    d_c2b = din("c2b", (32, 1), F32)
    d_w3 = din("w3", (3, 96, 32), BF16)
    d_c3b = din("c3b", (32, 1), F32)
    d_fcwp = din("fcwp", (12, 128, 256), BF16)
    d_fcwl = din("fcwl", (32, 256), BF16)
    d_fcb = din("fcb", (2, 128, 1), F32)
    d_ewih = din("ewih", (16, 1024), F8)
    d_ewhh = din("ewhh", (2, 128, 1024), F8)
    d_ebias = din("ebias", (8, 128, 1), F32)
    d_embt = din("embt", (2, 128, 32), BF16)
    d_embb = din("embb", (32, 1), F32)
    d_w0t = din("w0t", (3, 128, 2048), F8)
    d_wrt = din("wrt", (12, 128, 2048), F8)
    d_wht = din("wht", (16, 128, 2048), F8)
    d_b0 = din("b0", (16, 128, 1), F32)
    d_brep = din("brep", (3, 128, 64), F32)
    d_awct = din("awct", (4, 128, 9), BF16)
    d_awcb = din("awcb", (1, 9), BF16)
    d_out = nc.dram_tensor("out", [TBL, 3], F32, kind="ExternalOutput")

    C1 = CHUNK * 121
    C2 = CHUNK * 81
    C3 = CHUNK * 49

    with tile.TileContext(nc) as tc:
        with (
            tc.tile_pool(name="persist", bufs=1) as P,
            tc.tile_pool(name="io", bufs=2) as IO,
            tc.tile_pool(name="work", bufs=4) as W,
            tc.tile_pool(name="cpool", bufs=3) as CP,
            tc.tile_pool(name="ps", bufs=2, space="PSUM") as PS,
        ):
            # ---------- load persistent weights / tables ----------
            def ld(dram_ap, shape, dty, name):
                t = P.tile(shape, dty, tag=name)
                nc.sync.dma_start(t[:], dram_ap)
                return t

            w1 = P.tile([128, 64], BF16, tag="w1")
            for i in range(2):
                nc.sync.dma_start(w1[:, 32 * i:32 * i + 32], d_w1[i])
            c1b = ld(d_c1b[:], [32, 1], F32, "c1b")
            w2 = P.tile([96, 96], BF16, tag="w2")
            for j in range(3):
                nc.sync.dma_start(w2[:, 32 * j:32 * j + 32], d_w2[j])
            c2b = ld(d_c2b[:], [32, 1], F32, "c2b")
            w3 = P.tile([96, 96], BF16, tag="w3")
            for j in range(3):
                nc.sync.dma_start(w3[:, 32 * j:32 * j + 32], d_w3[j])
            c3b = ld(d_c3b[:], [32, 1], F32, "c3b")
            fcw = P.tile([128, 12 * 256], BF16, tag="fcw")
            for q in range(12):
                nc.sync.dma_start(fcw[:, 256 * q:256 * q + 256], d_fcwp[q])
            fcwl = ld(d_fcwl[:], [32, 256], BF16, "fcwl")
            fcb = P.tile([128, 2], F32, tag="fcb")
            for i in range(2):
                nc.sync.dma_start(fcb[:, i:i + 1], d_fcb[i])
            ewih = ld(d_ewih[:], [16, 1024], F8, "ewih")
            ewhh = P.tile([128, 2048], F8, tag="ewhh")
            for i in range(2):
                nc.sync.dma_start(ewhh[:, 1024 * i:1024 * i + 1024], d_ewhh[i])
            ebias = P.tile([128, 8], F32, tag="ebias")
            for i in range(8):
                nc.sync.dma_start(ebias[:, i:i + 1], d_ebias[i])
            embt = P.tile([128, 64], BF16, tag="embt")
            for i in range(2):
                nc.sync.dma_start(embt[:, 32 * i:32 * i + 32], d_embt[i])
            embb = ld(d_embb[:], [32, 1], F32, "embb")
            w0 = P.tile([128, 3 * 2048], F8, tag="w0")
            for i in range(3):
                nc.sync.dma_start(w0[:, 2048 * i:2048 * i + 2048], d_w0t[i])
            wr = P.tile([128, 12 * 2048], F8, tag="wr")
            for i in range(12):
                nc.sync.dma_start(wr[:, 2048 * i:2048 * i + 2048], d_wrt[i])
            wh = P.tile([128, 16 * 2048], F8, tag="wh")
            for i in range(16):
                nc.sync.dma_start(wh[:, 2048 * i:2048 * i + 2048], d_wht[i])
            b0 = P.tile([128, 16], F32, tag="b0")
            for i in range(16):
                nc.sync.dma_start(b0[:, i:i + 1], d_b0[i])
            brep = P.tile([128, 3 * 64], F32, tag="brep")
            for i in range(3):
                nc.sync.dma_start(brep[:, 64 * i:64 * i + 64], d_brep[i])
            awct = P.tile([128, 36], BF16, tag="awct")
            for i in range(4):
                nc.sync.dma_start(awct[:, 9 * i:9 * i + 9], d_awct[i])
            awcb = ld(d_awcb[:], [1, 9], BF16, "awcb")
            ones1 = P.tile([1, 128], BF16, tag="ones1")
            nc.gpsimd.memset(ones1[:], 1.0)
            oh = P.tile([128, 16], F32, tag="oh")
            for i in range(2):
                nc.sync.dma_start(oh[:, 8 * i:8 * i + 8], d_oh[i])
            langm = ld(d_langm[:], [16, TBL], BF16, "langm")

            # done masks: (1-done) broadcast to 128 partitions via ones-matmul,
            # then replicated per hidden-tile.
            drow = P.tile([1, TBL], F32, tag="drow")
            nc.sync.dma_start(drow[:], d_done[:])
            drow_i = P.tile([1, TBL], BF16, tag="drow_i")
            nc.scalar.activation(drow_i[:], drow[:], AF.Copy, scale=-1.0, bias=1.0)
            dbc_ps = PS.tile([128, TBL], F32, tag="pre")
            nc.tensor.matmul(dbc_ps[:], ones1[:, :], drow_i[:, :], start=True, stop=True)
            dbc = P.tile([128, TBL], BF16, tag="dbc")
            nc.scalar.activation(dbc[:], dbc_ps[:], AF.Copy)
            dbc_v = dbc[:].rearrange("p (t b) -> p t b", b=BL)
            donem_l = P.tile([128, 8 * T], BF16, tag="donem_l")
            dl_v = donem_l[:].rearrange("p (t x) -> p t x", x=8)
            for j in range(2):
                nc.vector.tensor_copy(dl_v[:, :, 4 * j:4 * j + 4], dbc_v)
            donem_m = P.tile([128, 16 * T], BF16, tag="donem_m")
            dm_v = donem_m[:].rearrange("p (t x) -> p t x", x=16)
            for j in range(4):
                nc.vector.tensor_copy(dm_v[:, :, 4 * j:4 * j + 4], dbc_v)

            # persistent activations
            Zl = P.tile([128, 8 * TBL], F32, tag="Zl")
            Z0 = P.tile([128, 16 * TBL], BF16, tag="Z0")
            langH = P.tile([128, 2 * TBL], BF16, tag="langH")
            imgh = P.tile([128, 2 * TBL], BF16, tag="imgh")
            lange = P.tile([32, TBL], BF16, tag="lange")
            memH = P.tile([128, 4 * TBL], BF16, tag="memH")

            # ---------- lang LSTM: precompute input gates ----------
            for m in range(8):
                zp = PS.tile([128, TBL], F32, tag="pre")
                nc.tensor.matmul(zp[:], ewih[:, 128 * m:128 * m + 128],
                                 langm[:], start=True, stop=True)
                nc.scalar.activation(Zl[:, TBL * m:TBL * (m + 1)], zp[:],
                                     AF.Identity, bias=ebias[:, m:m + 1])

            # ---------- lang LSTM scan ----------
            eh = CP.tile([128, 8], BF16, tag="eh")
            ec = CP.tile([128, 8], F32, tag="ec")
            for i in range(2):
                nc.sync.dma_start(eh[:, 4 * i:4 * i + 4], d_eh0[i])
                nc.sync.dma_start(ec[:, 4 * i:4 * i + 4], d_ec0[i])
            Zl_v = Zl[:].rearrange("p (m n) -> p m n", n=TBL)
            for t in range(T_SCAN):
                hm = W.tile([128, 8], BF16, tag="ehm")
                cm = W.tile([128, 8], F32, tag="ecm")
                dsl = donem_l[:, 8 * t:8 * t + 8]
                nc.vector.tensor_mul(hm[:], eh[:], dsl)
                nc.vector.tensor_mul(cm[:], ec[:], dsl)
                lg = PS.tile([128, 32], F32, tag="lg")
                for m in range(8):
                    for kt in range(2):
                        nc.tensor.matmul(lg[:, 4 * m:4 * m + 4],
                                         ewhh[:, 1024 * kt + 128 * m:1024 * kt + 128 * m + 128],
                                         hm[:, 4 * kt:4 * kt + 4],
                                         start=(kt == 0), stop=(kt == 1))
                gl = W.tile([128, 32], F32, tag="gl")
                nc.vector.tensor_add(gl[:].rearrange("p (m n) -> p m n", n=4),
                                     lg[:].rearrange("p (m n) -> p m n", n=4),
                                     Zl_v[:, :, 4 * t:4 * t + 4])
                sif = W.tile([128, 16], F32, tag="esif")
                tg = W.tile([128, 8], F32, tag="etg")
                so = W.tile([128, 8], F32, tag="eso")
                nc.scalar.activation(sif[:], gl[:, 0:16], AF.Sigmoid, scale=INV_SCALE)
                nc.scalar.activation(tg[:], gl[:, 16:24], AF.Tanh, scale=INV_SCALE)
                nc.scalar.activation(so[:], gl[:, 24:32], AF.Sigmoid, scale=INV_SCALE)
                t1 = W.tile([128, 8], F32, tag="et1")
                t2 = W.tile([128, 8], F32, tag="et2")
                nc.vector.tensor_mul(t1[:], sif[:, 8:16], cm[:])
                nc.vector.tensor_mul(t2[:], sif[:, 0:8], tg[:])
                ec = CP.tile([128, 8], F32, tag="ec")
                nc.vector.tensor_add(ec[:], t1[:], t2[:])
                th = W.tile([128, 8], F32, tag="eth")
                nc.scalar.activation(th[:], ec[:], AF.Tanh)
                eh = CP.tile([128, 8], BF16, tag="eh")
                nc.vector.tensor_mul(eh[:], so[:], th[:])
                nc.vector.tensor_copy(
                    langH[:].rearrange("p (j n) -> p j n", n=TBL)[:, :, BL * t:BL * t + BL],
                    eh[:].rearrange("p (j b) -> p j b", b=BL))

            # ---------- conv encoder (chunked over images) ----------
            for ch in range(NCHUNK):
                ca = IO.tile([128, C1], BF16, tag="im2a")
                cb = IO.tile([128, C1], BF16, tag="im2b")
                nc.sync.dma_start(ca[:], d_im2col[0, :, C1 * ch:C1 * (ch + 1)])
                nc.sync.dma_start(cb[:], d_im2col[1, :, C1 * ch:C1 * (ch + 1)])
                x13 = IO.tile([96, C1 + 24], BF16, tag="x13")
                for ns in range(4):  # 4 imgs per matmul: N=484
                    c1p = PS.tile([32, 484], F32, tag="cv")
                    nc.tensor.matmul(c1p[:], w1[:, 0:32],
                                     ca[:, 484 * ns:484 * (ns + 1)], start=True, stop=False)
                    nc.tensor.matmul(c1p[:], w1[:, 32:64],
                                     cb[:, 484 * ns:484 * (ns + 1)], start=False, stop=True)
                    nc.scalar.activation(x13[0:32, 484 * ns:484 * (ns + 1)], c1p[:],
                                         AF.Relu, bias=c1b[:])
                x13v = x13[0:32, 0:C1].rearrange("p (g c) -> p g c", c=121)
                for d, off in ((1, 11), (2, 22)):
                    nc.vector.tensor_copy(
                        x13[32 * d:32 * d + 32, 0:C1].rearrange(
                            "p (g c) -> p g c", c=121)[:, :, 0:121 - off],
                        x13v[:, :, off:121])
                x23 = IO.tile([96, C2 + 18], BF16, tag="x23")
                x13w = x13[:, 0:C1].rearrange("p (g a b) -> p g a b", a=11, b=11)
                for ns in range(4):  # 4 imgs per matmul: N=324
                    c2p = PS.tile([32, 4, 9, 9], F32, tag="cv")
                    for j in range(3):
                        nc.tensor.matmul(c2p[:], w2[:, 32 * j:32 * j + 32],
                                         x13w[:, 4 * ns:4 * ns + 4, 0:9, j:j + 9],
                                         start=(j == 0), stop=(j == 2))
                    nc.scalar.activation(
                        x23[0:32, 324 * ns:324 * (ns + 1)],
                        c2p[:].rearrange("p g a b -> p (g a b)"), AF.Relu, bias=c2b[:])
                x23v = x23[0:32, 0:C2].rearrange("p (g c) -> p g c", c=81)
                for d, off in ((1, 9), (2, 18)):
                    nc.vector.tensor_copy(
                        x23[32 * d:32 * d + 32, 0:C2].rearrange(
                            "p (g c) -> p g c", c=81)[:, :, 0:81 - off],
                        x23v[:, :, off:81])
                x34 = IO.tile([128, C3 + 3], BF16, tag="x34")
                x23w = x23[:, 0:C2].rearrange("p (g a b) -> p g a b", a=9, b=9)
                for ns in range(4):  # 4 imgs per matmul: N=196
                    c3p = PS.tile([32, 4, 7, 7], F32, tag="cv")
                    for j in range(3):
                        nc.tensor.matmul(c3p[:], w3[:, 32 * j:32 * j + 32],
                                         x23w[:, 4 * ns:4 * ns + 4, 0:7, j:j + 7],
                                         start=(j == 0), stop=(j == 2))
                    nc.scalar.activation(
                        x34[0:32, 196 * ns:196 * (ns + 1)],
                        c3p[:].rearrange("p g a b -> p (g a b)"), AF.Relu, bias=c3b[:])
                x34v = x34[0:32, 0:C3].rearrange("p (g c) -> p g c", c=49)
                for d in (1, 2, 3):
                    nc.vector.tensor_copy(
                        x34[32 * d:32 * d + 32, 0:C3].rearrange(
                            "p (g c) -> p g c", c=49)[:, :, 0:49 - d],
                        x34v[:, :, d:49])
                x34w = x34[:, 0:C3].rearrange("p (g c) -> p g c", c=49)
                for mt in range(2):
                    fp = PS.tile([128, CHUNK], F32, tag="cv")
                    for q in range(12):
                        nc.tensor.matmul(fp[:],
                                         fcw[:, 256 * q + 128 * mt:256 * q + 128 * mt + 128],
                                         x34w[:, :, 4 * q:4 * q + 1].opt(),
                                         start=(q == 0), stop=False)
                    nc.tensor.matmul(fp[:], fcwl[:, 128 * mt:128 * mt + 128],
                                     x34w[0:32, :, 48:49].opt(), start=False, stop=True)
                    nc.scalar.activation(
                        imgh[:, TBL * mt + CHUNK * ch:TBL * mt + CHUNK * (ch + 1)],
                        fp[:], AF.Relu, bias=fcb[:, mt:mt + 1])

            # ---------- lang embedding ----------
            ep = PS.tile([32, TBL], F32, tag="pre")
            for kt in range(2):
                nc.tensor.matmul(ep[:], embt[:, 32 * kt:32 * kt + 32],
                                 langH[:, TBL * kt:TBL * (kt + 1)],
                                 start=(kt == 0), stop=(kt == 1))
            nc.scalar.activation(lange[:], ep[:], AF.Relu, bias=embb[:])

            # ---------- mem LSTM: precompute layer-0 input gates ----------
            for m in range(16):
                z0p = PS.tile([128, TBL], F32, tag="pre")
                nc.tensor.matmul(z0p[:], w0[:, 128 * m:128 * m + 128],
                                 imgh[:, 0:TBL], start=True, stop=False)
                nc.tensor.matmul(z0p[:], w0[:, 2048 + 128 * m:2048 + 128 * m + 128],
                                 imgh[:, TBL:2 * TBL], start=False, stop=False)
                nc.tensor.matmul(z0p[:], w0[0:32, 4096 + 128 * m:4096 + 128 * m + 128],
                                 lange[:], start=False, stop=True)
                nc.scalar.activation(Z0[:, TBL * m:TBL * (m + 1)], z0p[:],
                                     AF.Identity, bias=b0[:, m:m + 1])

            # ---------- mem LSTM scan ----------
            mh = []
            mc = []
            for l in range(MEM_L):
                h_ = CP.tile([128, 16], BF16, tag=f"mh{l}")
                c_ = CP.tile([128, 16], F32, tag=f"mc{l}")
                for i in range(4):
                    nc.sync.dma_start(h_[:, 4 * i:4 * i + 4], d_mh0[l, i])
                    nc.sync.dma_start(c_[:, 4 * i:4 * i + 4], d_mc0[l, i])
                mh.append(h_)
                mc.append(c_)
            Z0_v = Z0[:].rearrange("p (m n) -> p m n", n=TBL)
            brep_v = brep[:].rearrange("p (l m n) -> p l m n", l=3, n=4)
            for t in range(T_SCAN):
                dsl = donem_m[:, 16 * t:16 * t + 16]
                xin = None
                for l in range(MEM_L):
                    hmm = W.tile([128, 16], BF16, tag=f"hm{l}")
                    cmm = W.tile([128, 16], F32, tag=f"cm{l}")
                    nc.vector.tensor_mul(hmm[:], mh[l][:], dsl)
                    nc.vector.tensor_mul(cmm[:], mc[l][:], dsl)
                    gp = PS.tile([128, 64], F32, tag="gp")
                    for m in range(16):
                        if l > 0:
                            base = ((l - 1) * 4) * 2048
                            for kt in range(4):
                                nc.tensor.matmul(
                                    gp[:, 4 * m:4 * m + 4],
                                    wr[:, base + kt * 2048 + 128 * m:base + kt * 2048 + 128 * m + 128],
                                    xin[:, 4 * kt:4 * kt + 4],
                                    start=(kt == 0), stop=False)
                        base = (l * 4) * 2048
                        for kt in range(4):
                            nc.tensor.matmul(
                                gp[:, 4 * m:4 * m + 4],
                                wh[:, base + kt * 2048 + 128 * m:base + kt * 2048 + 128 * m + 128],
                                hmm[:, 4 * kt:4 * kt + 4],
                                start=(l == 0 and kt == 0), stop=(kt == 3))
                    gs = W.tile([128, 64], F32, tag="gs")
                    if l == 0:
                        nc.vector.tensor_add(gs[:].rearrange("p (m n) -> p m n", n=4),
                                             gp[:].rearrange("p (m n) -> p m n", n=4),
                                             Z0_v[:, :, 4 * t:4 * t + 4])
                    else:
                        nc.vector.tensor_add(gs[:].rearrange("p (m n) -> p m n", n=4),
                                             gp[:].rearrange("p (m n) -> p m n", n=4),
                                             brep_v[:, l - 1])
                    sif = W.tile([128, 32], F32, tag="msif")
                    tg = W.tile([128, 16], F32, tag="mtg")
                    so = W.tile([128, 16], F32, tag="mso")
                    nc.scalar.activation(sif[:], gs[:, 0:32], AF.Sigmoid, scale=INV_SCALE)
                    nc.scalar.activation(tg[:], gs[:, 32:48], AF.Tanh, scale=INV_SCALE)
                    nc.scalar.activation(so[:], gs[:, 48:64], AF.Sigmoid, scale=INV_SCALE)
                    t1 = W.tile([128, 16], F32, tag=f"mt1{l}")
                    t2 = W.tile([128, 16], F32, tag=f"mt2{l}")
                    nc.vector.tensor_mul(t1[:], sif[:, 16:32], cmm[:])
                    nc.vector.tensor_mul(t2[:], sif[:, 0:16], tg[:])
                    c_ = CP.tile([128, 16], F32, tag=f"mc{l}")
                    nc.vector.tensor_add(c_[:], t1[:], t2[:])
                    mc[l] = c_
                    th = W.tile([128, 16], F32, tag=f"mth{l}")
                    nc.scalar.activation(th[:], c_[:], AF.Tanh)
                    h_ = CP.tile([128, 16], BF16, tag=f"mh{l}")
                    nc.vector.tensor_mul(h_[:], so[:], th[:])
                    mh[l] = h_
                    xin = h_
                nc.vector.tensor_copy(
                    memH[:].rearrange("p (j n) -> p j n", n=TBL)[:, :, BL * t:BL * t + BL],
                    mh[3][:].rearrange("p (j b) -> p j b", b=BL))

            # ---------- heads ----------
            for it in range(2):
                hd = PS.tile([128, 9], F32, tag="cv")
                for kt in range(4):
                    nc.tensor.matmul(hd[:],
                                     memH[:, TBL * kt + 128 * it:TBL * kt + 128 * it + 128],
                                     awct[:, 9 * kt:9 * kt + 9],
                                     start=(kt == 0), stop=False)
                nc.tensor.matmul(hd[:], ones1[:, :], awcb[:, :], start=False, stop=True)
                mx = W.tile([128, 1], F32, tag="hmx")
                nc.vector.reduce_max(mx[:], hd[:, 0:8], axis=AX.X)
                xm = W.tile([128, 8], F32, tag="hxm")
                nc.vector.tensor_scalar_sub(xm[:], hd[:, 0:8], mx[:])
                ex = W.tile([128, 8], F32, tag="hex")
                se = W.tile([128, 1], F32, tag="hse")
                nc.scalar.activation(ex[:], xm[:], AF.Exp, accum_out=se[:])
                lnv = W.tile([128, 1], F32, tag="hln")
                nc.scalar.activation(lnv[:], se[:], AF.Ln)
                logp = W.tile([128, 8], F32, tag="hlp")
                nc.vector.tensor_scalar_sub(logp[:], xm[:], lnv[:])
                lp1 = W.tile([128, 8], F32, tag="hlp1")
                nc.vector.tensor_mul(lp1[:], logp[:], oh[:, 8 * it:8 * it + 8])
                lpa = W.tile([128, 1], F32, tag="hlpa")
                nc.vector.reduce_sum(lpa[:], lp1[:], axis=AX.X)
                t3 = W.tile([128, 8], F32, tag="ht3")
                nc.vector.tensor_mul(t3[:], ex[:], xm[:])
                sxm = W.tile([128, 1], F32, tag="hsxm")
                nc.vector.reduce_sum(sxm[:], t3[:], axis=AX.X)
                rse = W.tile([128, 1], F32, tag="hrse")
                nc.vector.reciprocal(rse[:], se[:])
                m1 = W.tile([128, 1], F32, tag="hm1")
                nc.vector.tensor_mul(m1[:], sxm[:], rse[:])
                ent = W.tile([128, 1], F32, tag="hent")
                nc.vector.tensor_sub(ent[:], lnv[:], m1[:])
                osb = W.tile([128, 3], F32, tag="osb")
                nc.vector.tensor_copy(osb[:, 0:1], lpa[:])
                nc.vector.tensor_copy(osb[:, 1:2], ent[:])
                nc.vector.tensor_copy(osb[:, 2:3], hd[:, 8:9])
                nc.sync.dma_start(d_out[128 * it:128 * (it + 1), :], osb[:])

    nc.compile()
    return nc


def _prep_shared(inputs):
    f32 = np.float32
    out = {}
    w1p = np.concatenate([inputs["conv1_w"].reshape(16, 243).T.astype(f32),
                          np.zeros((13, 16), f32)], 0)          # (256,16)
    out["w1"] = np.concatenate([w1p, w1p], 1).reshape(2, 128, 32).astype(bf16)
    out["c1b"] = np.concatenate([inputs["conv1_b"]] * 2).reshape(32, 1).astype(f32)
    w2z = np.zeros((3, 96, 32), f32)
    for j in range(3):
        for d in range(3):
            w2z[j, 32 * d:32 * d + 16, :] = inputs["conv2_w"][:, :, d, j].T
    out["w2"] = w2z.astype(bf16)
    out["c2b"] = inputs["conv2_b"].reshape(32, 1).astype(f32)
    w3 = np.stack([inputs["conv3_w"][:, :, d, :] for d in range(3)])  # (3,32,32,3)
    out["w3"] = np.ascontiguousarray(
        w3.transpose(3, 0, 2, 1).reshape(3, 96, 32)).astype(bf16)
    out["c3b"] = inputs["conv3_b"].reshape(32, 1).astype(f32)
    F = inputs["fc_w"].reshape(256, 32, 49)
    fcwp = np.stack([
        np.ascontiguousarray(F[:, :, 4 * q:4 * q + 4].transpose(2, 1, 0)).reshape(128, 256)
        for q in range(12)])
    out["fcwp"] = fcwp.astype(bf16)
    out["fcwl"] = np.ascontiguousarray(F[:, :, 48].T).astype(bf16)
    out["fcb"] = inputs["fc_b"].reshape(2, 128, 1).astype(f32)
    out["ewih"] = (np.concatenate(
        [inputs["enc_Wih"].T.astype(f32), np.zeros((2, 1024), f32)], 0)
        * FP8_SCALE).astype(f8)
    out["ewhh"] = (np.ascontiguousarray(
        inputs["enc_Whh"].T.reshape(2, 128, 1024)) * FP8_SCALE).astype(f8)
    out["ebias"] = ((inputs["enc_bih"] + inputs["enc_bhh"])
                    * FP8_SCALE).reshape(8, 128, 1).astype(f32)
    out["embt"] = np.ascontiguousarray(
        inputs["emb_w"].T.reshape(2, 128, 32)).astype(bf16)
    out["embb"] = inputs["emb_b"].reshape(32, 1).astype(f32)
    w0 = np.concatenate([inputs["mem_Wih0"].T.astype(f32),
                         np.zeros((96, 2048), f32)], 0)  # pad 288->384
    out["w0t"] = (w0.reshape(3, 128, 2048) * FP8_SCALE).astype(f8)
    out["wrt"] = (np.ascontiguousarray(
        inputs["mem_WihR"].transpose(0, 2, 1).reshape(12, 128, 2048))
        * FP8_SCALE).astype(f8)
    out["wht"] = (np.ascontiguousarray(
        inputs["mem_Whh"].transpose(0, 2, 1).reshape(16, 128, 2048))
        * FP8_SCALE).astype(f8)
    bias = ((inputs["mem_bih"] + inputs["mem_bhh"]) * FP8_SCALE).astype(f32)
    out["b0"] = np.ascontiguousarray(bias[0].reshape(16, 128, 1))
    out["brep"] = np.ascontiguousarray(np.repeat(
        bias[1:].reshape(3, 16, 128).transpose(0, 2, 1), BL, axis=2))  # (3,128,64)
    out["awct"] = np.ascontiguousarray(np.concatenate(
        [inputs["actor_w"], inputs["critic_w"]], 0).T.reshape(4, 128, 9)).astype(bf16)
    out["awcb"] = np.concatenate(
        [inputs["actor_b"], inputs["critic_b"]]).reshape(1, 9).astype(bf16)
    return out


def _prep_core(inputs, k):
    f32 = np.float32
    out = {}
    img = np.asarray(inputs["img"], f32).reshape(T, B, 3, 11, 9, 11, 9)
    imk = img[:, BL * k:BL * (k + 1)]                      # (64,4,3,11,9,11,9)
    im2 = np.ascontiguousarray(
        imk.transpose(2, 4, 6, 0, 1, 3, 5)).reshape(243, TBL * 121)
    im2 = np.concatenate([im2, np.zeros((13, TBL * 121), f32)], 0)
    out["im2col"] = im2.reshape(2, 128, TBL * 121).astype(bf16)
    lk = np.asarray(inputs["lang"], f32)[:, BL * k:BL * (k + 1)]   # (64,4,14)
    lm = np.ascontiguousarray(lk.transpose(2, 0, 1)).reshape(14, TBL)
    out["langm"] = np.concatenate([lm, np.zeros((2, TBL), f32)], 0).astype(bf16)
    out["donerow"] = np.ascontiguousarray(
        np.asarray(inputs["done"], f32)[:, BL * k:BL * (k + 1)]).reshape(1, TBL)
    act = np.asarray(inputs["action"]).reshape(T, B)[:, BL * k:BL * (k + 1)].reshape(TBL)
    ohm = np.zeros((TBL, NACT), f32)
    ohm[np.arange(TBL), act.astype(np.int64)] = 1.0
    out["oh"] = ohm.reshape(2, 128, NACT)
    eh0 = np.ascontiguousarray(
        np.asarray(inputs["enc_h0"], f32)[BL * k:BL * (k + 1)].T)   # (256,4)
    ec0 = np.ascontiguousarray(
        np.asarray(inputs["enc_c0"], f32)[BL * k:BL * (k + 1)].T)
    out["eh0"] = eh0.reshape(2, 128, BL).astype(bf16)
    out["ec0"] = ec0.reshape(2, 128, BL)
    mh0 = np.ascontiguousarray(
        np.asarray(inputs["mem_h0"], f32)[:, BL * k:BL * (k + 1)].transpose(0, 2, 1))
    mc0 = np.ascontiguousarray(
        np.asarray(inputs["mem_c0"], f32)[:, BL * k:BL * (k + 1)].transpose(0, 2, 1))
    out["mh0"] = mh0.reshape(MEM_L, 4, 128, BL).astype(bf16)
    out["mc0"] = mc0.reshape(MEM_L, 4, 128, BL)
    return out


def kernel(**inputs):
    from concourse import bass_utils

    if "nc" not in _cache:
        _cache["nc"] = _build_nc()
    nc = _cache["nc"]

    shared = _prep_shared({k: np.asarray(v) for k, v in inputs.items()
                           if k not in ("img", "lang", "done", "action",
                                        "enc_h0", "enc_c0", "mem_h0", "mem_c0")})
    in_maps = []
    for k in range(NCORES):
        m = dict(shared)
        m.update(_prep_core(inputs, k))
        in_maps.append(m)

    res = bass_utils.run_bass_kernel_spmd(nc, in_maps, core_ids=list(range(NCORES)),
                                          trace=bool(int(os.environ.get("KERNEL_TRACE", "0"))))
    out_full = np.zeros((T, B, 3), np.float32)
    for k in range(NCORES):
        out_full[:, BL * k:BL * (k + 1)] = res.results[k]["out"].reshape(T, BL, 3)
    if os.environ.get("KERNEL_RESULT_STASH"):
        _cache["last_res"] = res
    return out_full.reshape(T * B, 3)



# revision 9
# speedup vs baseline: 1.5469x; 1.5303x over previous
"""Trainium2 Bass kernel for nn_Agent (conv encoder + masked LSTM scans + heads).

Sharding: data-parallel over batch B=32 across 8 cores (B_local=4). The
sequential T=64 scans run locally per core.

Structure (single fused program per core):
  - conv encoder chunks interleaved with the lang-LSTM scan steps (fills PE
    gaps left by the lang recurrence's activation chain).
  - mem LSTM restructured as a chunk-wavefront: layer l processes T-chunk k
    after layer l-1 finished chunk k. The input-side gates (W_ih @ x) are
    batched per chunk (N=64 moving columns) instead of per step (N=4),
    removing ~40% of the PE instructions from the sequential loop. Up to 4
    (layer, chunk) blocks are interleaved step-by-step so each block's
    activation chain hides under the other blocks' matmuls.
  - recurrent/input weights stored fp8(e4m3) scaled by 1024 (keeps values
    out of fp8-denormal range); the 1/1024 is folded into the sigmoid/tanh
    activation `scale` and the precomputed gate biases.
  - gates reordered (i,f,o | g) so one sigmoid ACT covers i,f,o.
"""
import os
import sys
import numpy as np

for p in ("/opt/trn_rl_repo",):
    if p not in sys.path:
        sys.path.insert(0, p)

import ml_dtypes

bf16 = ml_dtypes.bfloat16
f8 = ml_dtypes.float8_e4m3
FP8_SCALE = 1024.0          # keep fp8-stored weights out of denormal range
INV_SCALE = 1.0 / FP8_SCALE

T, B = 64, 32
NCORES = 8
BL = B // NCORES            # 4 envs per core
TBL = T * BL                # 256 images per core
LANG_DIM, ENC_H, EMB_D, MEM_H, MEM_L, MEM_IN, NACT = 14, 256, 32, 512, 4, 288, 8

CHUNK = 16                  # conv pipeline: images per chunk
NCHUNK = TBL // CHUNK       # 16 conv chunks
TC = 16                     # scan chunk (timesteps)
NTC = T // TC               # 4 scan chunks
CB = TC * BL                # 64 moving columns per scan-chunk batch

# gate m-tile permutations: [i, f, o, g] blocks
PERM16 = [0, 1, 2, 3, 4, 5, 6, 7, 12, 13, 14, 15, 8, 9, 10, 11]
PERM8 = [0, 1, 2, 3, 6, 7, 4, 5]

_cache = {}


def _build_nc():
    import concourse.bacc as bacc
    import concourse.tile as tile
    from concourse import mybir

    dt = mybir.dt
    AF = mybir.ActivationFunctionType
    AX = mybir.AxisListType
    F32, BF16, F8 = dt.float32, dt.bfloat16, dt.float8e4

    nc = bacc.Bacc("TRN2", target_bir_lowering=False, debug=False,
                   enable_asserts=False, num_devices=NCORES)

    def din(name, shape, dty):
        return nc.dram_tensor(name, list(shape), dty, kind="ExternalInput")

    # ---------------- DRAM I/O ----------------
    d_im2col = din("im2col", (2, 128, TBL * 121), BF16)   # K-tiles x part x cols
    d_langm = din("langm", (16, TBL), BF16)
    d_done = din("donerow", (1, TBL), F32)
    d_oh = din("oh", (2, 128, NACT), F32)
    d_eh0 = din("eh0", (2, 128, BL), BF16)
    d_ec0 = din("ec0", (2, 128, BL), F32)
    d_mh0 = din("mh0", (MEM_L, 4, 128, BL), BF16)
    d_mc0 = din("mc0", (MEM_L, 4, 128, BL), F32)
    d_w1 = din("w1", (2, 128, 32), BF16)
    d_c1b = din("c1b", (32, 1), F32)
    d_w2 = din("w2", (3, 96, 32), BF16)
    d_c2b = din("c2b", (32, 1), F32)
    d_w3 = din("w3", (3, 96, 32), BF16)
    d_c3b = din("c3b", (32, 1), F32)
    d_fcwp = din("fcwp", (12, 128, 256), BF16)
    d_fcwl = din("fcwl", (32, 256), BF16)
    d_fcb = din("fcb", (2, 128, 1), F32)
    d_ewih = din("ewih", (16, 1024), F8)
    d_ewhh = din("ewhh", (2, 128, 1024), F8)
    d_ebias = din("ebias", (8, 128, 1), F32)
    d_embt = din("embt", (2, 128, 32), BF16)
    d_embb = din("embb", (32, 1), F32)
    d_w0t = din("w0t", (3, 128, 2048), F8)
    d_wrt = din("wrt", (12, 128, 2048), F8)
    d_wht = din("wht", (16, 128, 2048), F8)
    d_b0 = din("b0", (16, 128, 1), F32)
    d_brep = din("brep", (3, 128, 16), F32)
    d_awct = din("awct", (4, 128, 9), BF16)
    d_awcb = din("awcb", (1, 9), BF16)
    d_out = nc.dram_tensor("out", [TBL, 3], F32, kind="ExternalOutput")

    C1 = CHUNK * 121
    C2 = CHUNK * 81
    C3 = CHUNK * 49

    with tile.TileContext(nc) as tc:
        with (
            tc.tile_pool(name="persist", bufs=1) as P,
            tc.tile_pool(name="io", bufs=2) as IO,
            tc.tile_pool(name="work", bufs=4) as W,
            tc.tile_pool(name="xbuf", bufs=2) as XB,
            tc.tile_pool(name="cpool", bufs=3) as CP,
            tc.tile_pool(name="ps", bufs=2, space="PSUM") as PS,
            tc.tile_pool(name="psb", bufs=2, space="PSUM") as PSB,
        ):
            # ---------- persistent weights / tables ----------
            def ld(dram_ap, shape, dty, name):
                t = P.tile(shape, dty, tag=name)
                nc.sync.dma_start(t[:], dram_ap)
                return t

            def ldm(dram_t, tile_t, nblk, blkw):
                nc.sync.dma_start(
                    tile_t[:].rearrange("p (i c) -> p i c", i=nblk),
                    dram_t[:].rearrange("i p c -> p i c"))

            # lang encoder weights first (needed immediately)
            langm = ld(d_langm[:], [16, TBL], BF16, "langm")
            ewih = ld(d_ewih[:], [16, 1024], F8, "ewih")
            ewhh = P.tile([128, 2048], F8, tag="ewhh")
            nc.sync.dma_start(
                ewhh[:].rearrange("p (i c) -> p i c", i=2),
                d_ewhh[:].rearrange("i p c -> p i c"))
            ebias = P.tile([128, 8], F32, tag="ebias")
            ldm(d_ebias, ebias, 8, 1)
            drow = P.tile([1, TBL], F32, tag="drow")
            nc.sync.dma_start(drow[:], d_done[:])

            # conv weights
            w1 = P.tile([128, 64], BF16, tag="w1")
            ldm(d_w1, w1, 2, 32)
            c1b = ld(d_c1b[:], [32, 1], F32, "c1b")
            w2 = P.tile([96, 96], BF16, tag="w2")
            ldm(d_w2, w2, 3, 32)
            c2b = ld(d_c2b[:], [32, 1], F32, "c2b")
            w3 = P.tile([96, 96], BF16, tag="w3")
            ldm(d_w3, w3, 3, 32)
            c3b = ld(d_c3b[:], [32, 1], F32, "c3b")
            fcw = P.tile([128, 12 * 256], BF16, tag="fcw")
            ldm(d_fcwp, fcw, 12, 256)
            fcwl = ld(d_fcwl[:], [32, 256], BF16, "fcwl")
            fcb = P.tile([128, 2], F32, tag="fcb")
            ldm(d_fcb, fcb, 2, 1)
            embt = P.tile([128, 64], BF16, tag="embt")
            ldm(d_embt, embt, 2, 32)
            embb = ld(d_embb[:], [32, 1], F32, "embb")

            # mem LSTM weights: issued on the scalar hwdge ring so they do
            # not queue ahead of the im2col chunk stream on the sync ring.
            w0 = P.tile([128, 3 * 2048], F8, tag="w0")
            ldm(d_w0t, w0, 3, 2048)
            wr = P.tile([128, 12 * 2048], F8, tag="wr")
            ldm(d_wrt, wr, 12, 2048)
            wh = P.tile([128, 16 * 2048], F8, tag="wh")
            ldm(d_wht, wh, 16, 2048)
            b0 = P.tile([128, 16], F32, tag="b0")
            ldm(d_b0, b0, 16, 1)
            brep = P.tile([128, 3 * 16], F32, tag="brep")
            ldm(d_brep, brep, 3, 16)
            awct = P.tile([128, 36], BF16, tag="awct")
            ldm(d_awct, awct, 4, 9)
            awcb = ld(d_awcb[:], [1, 9], BF16, "awcb")
            oh = P.tile([128, 16], F32, tag="oh")
            ldm(d_oh, oh, 2, 8)
            ones1 = P.tile([1, 128], BF16, tag="ones1")
            nc.gpsimd.memset(ones1[:], 1.0)

            # ---------- state sequences ----------
            # langHs slot t holds lang-h_{t-1}; slot 0 = initial state.
            langHs = P.tile([128, (T + 1) * 8], BF16, tag="langHs")
            lhs_v = langHs[:].rearrange("p (t x) -> p t x", x=8)
            lhs_v4 = langHs[:].rearrange("p (t k b) -> p t k b", k=2, b=BL)
            nc.sync.dma_start(lhs_v4[:, 0],
                              d_eh0[:].rearrange("i p c -> p i c"))
            ec0 = CP.tile([128, 8], F32, tag="ec")
            nc.sync.dma_start(ec0[:].rearrange("p (i c) -> p i c", i=2),
                              d_ec0[:].rearrange("i p c -> p i c"))
            # hSeq[l] slot t holds mem-h_{l,t-1}
            hSeq = []
            hs_v = []
            hs_v4 = []
            for l in range(MEM_L):
                hs = P.tile([128, (T + 1) * 16], BF16, tag=f"hseq{l}")
                hSeq.append(hs)
                hs_v.append(hs[:].rearrange("p (t x) -> p t x", x=16))
                hs_v4.append(hs[:].rearrange("p (t k b) -> p t k b", k=4, b=BL))
                nc.sync.dma_start(hs_v4[l][:, 0],
                                  d_mh0[l].rearrange("i p c -> p i c"))
            mc = []
            for l in range(MEM_L):
                c_ = CP.tile([128, 16], F32, tag=f"mc{l}")
                nc.sync.dma_start(c_[:].rearrange("p (i c) -> p i c", i=4),
                                  d_mc0[l].rearrange("i p c -> p i c"))
                mc.append(c_)

            # done masks: (1-done) broadcast to 128 partitions via ones-matmul,
            # then replicated per hidden-tile.
            drow_i = P.tile([1, TBL], BF16, tag="drow_i")
            nc.scalar.activation(drow_i[:], drow[:], AF.Copy, scale=-1.0, bias=1.0)
            dbc_ps = PS.tile([128, TBL], F32, tag="pre")
            nc.tensor.matmul(dbc_ps[:], ones1[:, :], drow_i[:, :], start=True, stop=True)
            dbc = P.tile([128, TBL], BF16, tag="dbc")
            nc.scalar.activation(dbc[:], dbc_ps[:], AF.Copy)
            dbc_v = dbc[:].rearrange("p (t b) -> p t b", b=BL)
            donem_l = P.tile([128, 8 * T], BF16, tag="donem_l")
            dl_v = donem_l[:].rearrange("p (t x) -> p t x", x=8)
            for j in range(2):
                nc.vector.tensor_copy(dl_v[:, :, 4 * j:4 * j + 4], dbc_v)
            donem_m = P.tile([128, 16 * T], BF16, tag="donem_m")
            dm_v = donem_m[:].rearrange("p (t x) -> p t x", x=16)
            for j in range(4):
                nc.vector.tensor_copy(dm_v[:, :, 4 * j:4 * j + 4], dbc_v)

            # persistent activations
            Zl = P.tile([128, 8 * TBL], F32, tag="Zl")
            imgh = P.tile([128, 2 * TBL], BF16, tag="imgh")
            lange = P.tile([32, TBL], BF16, tag="lange")
            memH = P.tile([128, 4 * TBL], BF16, tag="memH")

            # ---------- lang LSTM: precompute input gates (scaled space) ----
            for m in range(8):
                zp = PS.tile([128, TBL], F32, tag="pre")
                nc.tensor.matmul(zp[:], ewih[:, 128 * m:128 * m + 128],
                                 langm[:], start=True, stop=True)
                nc.scalar.activation(Zl[:, TBL * m:TBL * (m + 1)], zp[:],
                                     AF.Identity, bias=ebias[:, m:m + 1])
            Zl_v = Zl[:].rearrange("p (m n) -> p m n", n=TBL)

            # ---------- lang scan step ----------
            ec_state = [ec0]

            def lang_step(t, lg):
                ec = ec_state[0]
                hm = W.tile([128, 8], BF16, tag="ehm")
                cm = W.tile([128, 8], F32, tag="ecm")
                dsl = donem_l[:, 8 * t:8 * t + 8]
                nc.vector.tensor_mul(hm[:], lhs_v[:, t], dsl)
                nc.vector.tensor_mul(cm[:], ec[:], dsl)
                for m in range(8):
                    for kt in range(2):
                        nc.tensor.matmul(lg[:, 4 * m:4 * m + 4],
                                         ewhh[:, 1024 * kt + 128 * m:1024 * kt + 128 * m + 128],
                                         hm[:, 4 * kt:4 * kt + 4],
                                         start=(kt == 0), stop=(kt == 1))
                gl = W.tile([128, 32], F32, tag="gl")
                nc.vector.tensor_add(gl[:].rearrange("p (m n) -> p m n", n=4),
                                     lg.rearrange("p (m n) -> p m n", n=4),
                                     Zl_v[:, :, 4 * t:4 * t + 4])
                sif = W.tile([128, 24], F32, tag="esif")
                tg = W.tile([128, 8], F32, tag="etg")
                nc.scalar.activation(sif[:], gl[:, 0:24], AF.Sigmoid, scale=INV_SCALE)
                nc.scalar.activation(tg[:], gl[:, 24:32], AF.Tanh, scale=INV_SCALE)
                t1 = W.tile([128, 8], F32, tag="et1")
                t2 = W.tile([128, 8], F32, tag="et2")
                nc.vector.tensor_mul(t1[:], sif[:, 8:16], cm[:])
                nc.vector.tensor_mul(t2[:], sif[:, 0:8], tg[:])
                ec = CP.tile([128, 8], F32, tag="ec")
                nc.vector.tensor_add(ec[:], t1[:], t2[:])
                ec_state[0] = ec
                th = W.tile([128, 8], F32, tag="eth")
                nc.scalar.activation(th[:], ec[:], AF.Tanh)
                nc.vector.tensor_mul(lhs_v[:, t + 1], sif[:, 16:24], th[:])

            # ---------- conv chunk (stages separated by hook calls) --------
            def conv_chunk(ch, hook):
                im = IO.tile([128, 2 * C1], BF16, tag="im2")
                im_v = im[:].rearrange("p (i c) -> p i c", i=2)
                nc.sync.dma_start(
                    im_v, d_im2col[:, :, C1 * ch:C1 * (ch + 1)].rearrange(
                        "i p c -> p i c"))
                ca = im_v[:, 0]
                cb = im_v[:, 1]
                x13 = IO.tile([96, C1 + 24], BF16, tag="x13")
                for ns in range(4):  # 4 imgs per matmul: N=484
                    c1p = PS.tile([32, 484], F32, tag="cv")
                    nc.tensor.matmul(c1p[:], w1[:, 0:32],
                                     ca[:, 484 * ns:484 * (ns + 1)], start=True, stop=False)
                    nc.tensor.matmul(c1p[:], w1[:, 32:64],
                                     cb[:, 484 * ns:484 * (ns + 1)], start=False, stop=True)
                    nc.scalar.activation(x13[0:32, 484 * ns:484 * (ns + 1)], c1p[:],
                                         AF.Relu, bias=c1b[:])
                hook(0)
                x13v = x13[0:32, 0:C1].rearrange("p (g c) -> p g c", c=121)
                for d, off in ((1, 11), (2, 22)):
                    nc.vector.tensor_copy(
                        x13[32 * d:32 * d + 32, 0:C1].rearrange(
                            "p (g c) -> p g c", c=121)[:, :, 0:121 - off],
                        x13v[:, :, off:121])
                x23 = IO.tile([96, C2 + 18], BF16, tag="x23")
                x13w = x13[:, 0:C1].rearrange("p (g a b) -> p g a b", a=11, b=11)
                for ns in range(4):  # 4 imgs per matmul: N=324
                    c2p = PS.tile([32, 4, 9, 9], F32, tag="cv")
                    for j in range(3):
                        nc.tensor.matmul(c2p[:], w2[:, 32 * j:32 * j + 32],
                                         x13w[:, 4 * ns:4 * ns + 4, 0:9, j:j + 9],
                                         start=(j == 0), stop=(j == 2))
                    nc.scalar.activation(
                        x23[0:32, 324 * ns:324 * (ns + 1)],
                        c2p[:].rearrange("p g a b -> p (g a b)"), AF.Relu, bias=c2b[:])
                hook(1)
                x23v = x23[0:32, 0:C2].rearrange("p (g c) -> p g c", c=81)
                for d, off in ((1, 9), (2, 18)):
                    nc.vector.tensor_copy(
                        x23[32 * d:32 * d + 32, 0:C2].rearrange(
                            "p (g c) -> p g c", c=81)[:, :, 0:81 - off],
                        x23v[:, :, off:81])
                x34 = IO.tile([128, C3 + 3], BF16, tag="x34")
                x23w = x23[:, 0:C2].rearrange("p (g a b) -> p g a b", a=9, b=9)
                for ns in range(4):  # 4 imgs per matmul: N=196
                    c3p = PS.tile([32, 4, 7, 7], F32, tag="cv")
                    for j in range(3):
                        nc.tensor.matmul(c3p[:], w3[:, 32 * j:32 * j + 32],
                                         x23w[:, 4 * ns:4 * ns + 4, 0:7, j:j + 7],
                                         start=(j == 0), stop=(j == 2))
                    nc.scalar.activation(
                        x34[0:32, 196 * ns:196 * (ns + 1)],
                        c3p[:].rearrange("p g a b -> p (g a b)"), AF.Relu, bias=c3b[:])
                hook(2)
                x34v = x34[0:32, 0:C3].rearrange("p (g c) -> p g c", c=49)
                for d in (1, 2, 3):
                    nc.vector.tensor_copy(
                        x34[32 * d:32 * d + 32, 0:C3].rearrange(
                            "p (g c) -> p g c", c=49)[:, :, 0:49 - d],
                        x34v[:, :, d:49])
                x34w = x34[:, 0:C3].rearrange("p (g c) -> p g c", c=49)
                for mt in range(2):
                    fp = PS.tile([128, CHUNK], F32, tag="cv")
                    for q in range(12):
                        nc.tensor.matmul(fp[:],
                                         fcw[:, 256 * q + 128 * mt:256 * q + 128 * mt + 128],
                                         x34w[:, :, 4 * q:4 * q + 1].opt(),
                                         start=(q == 0), stop=False)
                    nc.tensor.matmul(fp[:], fcwl[:, 128 * mt:128 * mt + 128],
                                     x34w[0:32, :, 48:49].opt(), start=False, stop=True)
                    nc.scalar.activation(
                        imgh[:, TBL * mt + CHUNK * ch:TBL * mt + CHUNK * (ch + 1)],
                        fp[:], AF.Relu, bias=fcb[:, mt:mt + 1])
                hook(3)

            # ---------- lang embedding per scan-chunk ----------
            def emb_chunk(k):
                ep = PS.tile([32, CB], F32, tag="pre")
                for kt in range(2):
                    nc.tensor.matmul(ep[:], embt[:, 32 * kt:32 * kt + 32],
                                     lhs_v4[:, 1 + TC * k:1 + TC * (k + 1), kt],
                                     start=(kt == 0), stop=(kt == 1))
                nc.scalar.activation(lange[:, CB * k:CB * (k + 1)], ep[:],
                                     AF.Relu, bias=embb[:])

            # ---------- mem scan: per-chunk input-gate batch ----------
            wrb = [None] * MEM_L

            def mem_batch(l, k):
                wb = XB.tile([128, 16 * CB], F32, tag=f"wrb{l}")
                wb_v = wb[:].rearrange("p (m c) -> p m c", c=CB)
                for half in range(2):
                    bp = PSB.tile([128, 8 * CB], F32, tag="bp")
                    bp_v = bp[:].rearrange("p (m c) -> p m c", c=CB)
                    for mm in range(8):
                        m = 8 * half + mm
                        if l == 0:
                            for kt in range(2):
                                nc.tensor.matmul(
                                    bp_v[:, mm],
                                    w0[:, 2048 * kt + 128 * m:2048 * kt + 128 * m + 128],
                                    imgh[:, TBL * kt + CB * k:TBL * kt + CB * (k + 1)],
                                    start=(kt == 0), stop=False)
                            nc.tensor.matmul(
                                bp_v[:, mm],
                                w0[0:32, 4096 + 128 * m:4096 + 128 * m + 128],
                                lange[:, CB * k:CB * (k + 1)],
                                start=False, stop=True)
                        else:
                            base = (l - 1) * 4 * 2048
                            for kt in range(4):
                                nc.tensor.matmul(
                                    bp_v[:, mm],
                                    wr[:, base + 2048 * kt + 128 * m:base + 2048 * kt + 128 * m + 128],
                                    hs_v4[l - 1][:, 1 + TC * k:1 + TC * (k + 1), kt],
                                    start=(kt == 0), stop=(kt == 3))
                        bias = b0[:, m:m + 1] if l == 0 else \
                            brep[:, 16 * (l - 1) + m:16 * (l - 1) + m + 1]
                        nc.scalar.activation(wb_v[:, m], bp_v[:, mm],
                                             AF.Identity, bias=bias)
                wrb[l] = wb[:].rearrange("p (m tl b) -> p m tl b", m=16, b=BL)

            # ---------- mem scan step ----------
            def mem_step(l, t, gp):
                dsl = donem_m[:, 16 * t:16 * t + 16]
                hm = W.tile([128, 16], BF16, tag=f"hm{l}")
                cm = W.tile([128, 16], F32, tag=f"cm{l}")
                nc.vector.tensor_mul(hm[:], hs_v[l][:, t], dsl)
                nc.vector.tensor_mul(cm[:], mc[l][:], dsl)
                base = l * 4 * 2048
                for m in range(16):
                    for kt in range(4):
                        nc.tensor.matmul(
                            gp[:, 4 * m:4 * m + 4],
                            wh[:, base + 2048 * kt + 128 * m:base + 2048 * kt + 128 * m + 128],
                            hm[:, 4 * kt:4 * kt + 4],
                            start=(kt == 0), stop=(kt == 3))
                gs = W.tile([128, 64], F32, tag=f"gs{l}")
                nc.vector.tensor_add(gs[:].rearrange("p (m n) -> p m n", n=4),
                                     gp.rearrange("p (m n) -> p m n", n=4),
                                     wrb[l][:, :, t % TC])
                sif = W.tile([128, 48], F32, tag=f"msif{l}")
                tg = W.tile([128, 16], F32, tag=f"mtg{l}")
                nc.scalar.activation(sif[:], gs[:, 0:48], AF.Sigmoid, scale=INV_SCALE)
                nc.scalar.activation(tg[:], gs[:, 48:64], AF.Tanh, scale=INV_SCALE)
                t1 = W.tile([128, 16], F32, tag=f"mt1{l}")
                t2 = W.tile([128, 16], F32, tag=f"mt2{l}")
                nc.vector.tensor_mul(t1[:], sif[:, 16:32], cm[:])
                nc.vector.tensor_mul(t2[:], sif[:, 0:16], tg[:])
                c_ = CP.tile([128, 16], F32, tag=f"mc{l}")
                nc.vector.tensor_add(c_[:], t1[:], t2[:])
                mc[l] = c_
                th = W.tile([128, 16], F32, tag=f"mth{l}")
                nc.scalar.activation(th[:], c_[:], AF.Tanh)
                nc.vector.tensor_mul(hs_v[l][:, t + 1], sif[:, 32:48], th[:])

            # copy hSeq[3] chunk into kt-major memH layout for the heads
            def memh_chunk(k):
                mH = memH[:].rearrange("p (kt t b) -> p kt t b", kt=4, b=BL)
                for kt in range(4):
                    nc.vector.tensor_copy(
                        mH[:, kt, TC * k:TC * (k + 1)],
                        hs_v4[3][:, 1 + TC * k:1 + TC * (k + 1), kt])

            # ---------- heads ----------
            def heads(it):
                hd = PS.tile([128, 9], F32, tag="pre")
                for kt in range(4):
                    nc.tensor.matmul(hd[:],
                                     memH[:, TBL * kt + 128 * it:TBL * kt + 128 * it + 128],
                                     awct[:, 9 * kt:9 * kt + 9],
                                     start=(kt == 0), stop=False)
                nc.tensor.matmul(hd[:], ones1[:, :], awcb[:, :], start=False, stop=True)
                mx = W.tile([128, 1], F32, tag="hmx")
                nc.vector.reduce_max(mx[:], hd[:, 0:8], axis=AX.X)
                xm = W.tile([128, 8], F32, tag="hxm")
                nc.vector.tensor_scalar_sub(xm[:], hd[:, 0:8], mx[:])
                ex = W.tile([128, 8], F32, tag="hex")
                se = W.tile([128, 1], F32, tag="hse")
                nc.scalar.activation(ex[:], xm[:], AF.Exp, accum_out=se[:])
                lnv = W.tile([128, 1], F32, tag="hln")
                nc.scalar.activation(lnv[:], se[:], AF.Ln)
                logp = W.tile([128, 8], F32, tag="hlp")
                nc.vector.tensor_scalar_sub(logp[:], xm[:], lnv[:])
                lp1 = W.tile([128, 8], F32, tag="hlp1")
                nc.vector.tensor_mul(lp1[:], logp[:], oh[:, 8 * it:8 * it + 8])
                lpa = W.tile([128, 1], F32, tag="hlpa")
                nc.vector.reduce_sum(lpa[:], lp1[:], axis=AX.X)
                t3 = W.tile([128, 8], F32, tag="ht3")
                nc.vector.tensor_mul(t3[:], ex[:], xm[:])
                sxm = W.tile([128, 1], F32, tag="hsxm")
                nc.vector.reduce_sum(sxm[:], t3[:], axis=AX.X)
                rse = W.tile([128, 1], F32, tag="hrse")
                nc.vector.reciprocal(rse[:], se[:])
                m1 = W.tile([128, 1], F32, tag="hm1")
                nc.vector.tensor_mul(m1[:], sxm[:], rse[:])
                ent = W.tile([128, 1], F32, tag="hent")
                nc.vector.tensor_sub(ent[:], lnv[:], m1[:])
                osb = W.tile([128, 3], F32, tag="osb")
                nc.vector.tensor_copy(osb[:, 0:1], lpa[:])
                nc.vector.tensor_copy(osb[:, 1:2], ent[:])
                nc.vector.tensor_copy(osb[:, 2:3], hd[:, 8:9])
                nc.sync.dma_start(d_out[128 * it:128 * (it + 1), :], osb[:])

            # ---------- unified slot schedule ----------
            # slot s: conv chunks 4s..4s+3 (s<4) + mem blocks (l, k=s-1-l)
            lang_t = [0]
            for s in range(NTC + MEM_L):
                active = [(l, s - 1 - l) for l in range(MEM_L)
                          if 0 <= s - 1 - l < NTC]
                for (l, k) in active:
                    mem_batch(l, k)

                def hook(q, hi):
                    hps = PS.tile([128, 320], F32, tag="hps")
                    if lang_t[0] < T and s < NTC:
                        lang_step(lang_t[0], hps[:, 256:288])
                        lang_t[0] += 1
                    for bi, (l, k) in enumerate(active):
                        mem_step(l, TC * k + 4 * q + hi,
                                 hps[:, 64 * bi:64 * bi + 64])

                for q in range(4):
                    if s < NTC:
                        conv_chunk(4 * s + q,
                                   lambda hi, q=q: hook(q, hi))
                    else:
                        for hi in range(4):
                            hook(q, hi)
                if s < NTC:
                    emb_chunk(s)
                for (l, k) in active:
                    if l == 3:
                        memh_chunk(k)
                        if k == 1:
                            heads(0)
                        if k == 3:
                            heads(1)

    nc.compile()
    return nc


def _prep_shared(inputs):
    f32 = np.float32

    def permg(w, nblk, perm):
        # permute gate blocks on the LAST axis
        s = w.shape
        v = w.reshape(s[:-1] + (nblk, s[-1] // nblk))
        return np.ascontiguousarray(v[..., perm, :]).reshape(s)

    out = {}
    w1p = np.concatenate([inputs["conv1_w"].reshape(16, 243).T.astype(f32),
                          np.zeros((13, 16), f32)], 0)          # (256,16)
    out["w1"] = np.concatenate([w1p, w1p], 1).reshape(2, 128, 32).astype(bf16)
    out["c1b"] = np.concatenate([inputs["conv1_b"]] * 2).reshape(32, 1).astype(f32)
    w2z = np.zeros((3, 96, 32), f32)
    for j in range(3):
        for d in range(3):
            w2z[j, 32 * d:32 * d + 16, :] = inputs["conv2_w"][:, :, d, j].T
    out["w2"] = w2z.astype(bf16)
    out["c2b"] = inputs["conv2_b"].reshape(32, 1).astype(f32)
    w3 = np.stack([inputs["conv3_w"][:, :, d, :] for d in range(3)])  # (3,32,32,3)
    out["w3"] = np.ascontiguousarray(
        w3.transpose(3, 0, 2, 1).reshape(3, 96, 32)).astype(bf16)
    out["c3b"] = inputs["conv3_b"].reshape(32, 1).astype(f32)
    F = inputs["fc_w"].reshape(256, 32, 49)
    fcwp = np.stack([
        np.ascontiguousarray(F[:, :, 4 * q:4 * q + 4].transpose(2, 1, 0)).reshape(128, 256)
        for q in range(12)])
    out["fcwp"] = fcwp.astype(bf16)
    out["fcwl"] = np.ascontiguousarray(F[:, :, 48].T).astype(bf16)
    out["fcb"] = inputs["fc_b"].reshape(2, 128, 1).astype(f32)
    ewih = permg(np.concatenate(
        [inputs["enc_Wih"].T.astype(f32), np.zeros((2, 1024), f32)], 0),
        8, PERM8)
    out["ewih"] = (ewih * FP8_SCALE).astype(f8)
    ewhh = permg(inputs["enc_Whh"].T.astype(f32), 8, PERM8)
    out["ewhh"] = (np.ascontiguousarray(
        ewhh.reshape(2, 128, 1024)) * FP8_SCALE).astype(f8)
    ebias = permg((inputs["enc_bih"] + inputs["enc_bhh"]).astype(f32), 8, PERM8)
    out["ebias"] = (ebias * FP8_SCALE).reshape(8, 128, 1).astype(f32)
    out["embt"] = np.ascontiguousarray(
        inputs["emb_w"].T.reshape(2, 128, 32)).astype(bf16)
    out["embb"] = inputs["emb_b"].reshape(32, 1).astype(f32)
    w0 = permg(np.concatenate([inputs["mem_Wih0"].T.astype(f32),
                               np.zeros((96, 2048), f32)], 0), 16, PERM16)
    out["w0t"] = (w0.reshape(3, 128, 2048) * FP8_SCALE).astype(f8)
    wrt = permg(np.ascontiguousarray(
        inputs["mem_WihR"].transpose(0, 2, 1)).astype(f32), 16, PERM16)
    out["wrt"] = (wrt.reshape(12, 128, 2048) * FP8_SCALE).astype(f8)
    wht = permg(np.ascontiguousarray(
        inputs["mem_Whh"].transpose(0, 2, 1)).astype(f32), 16, PERM16)
    out["wht"] = (wht.reshape(16, 128, 2048) * FP8_SCALE).astype(f8)
    bias = ((inputs["mem_bih"] + inputs["mem_bhh"]) * FP8_SCALE).astype(f32)
    b0p = bias[0].reshape(16, 128)[PERM16]
    out["b0"] = np.ascontiguousarray(b0p.reshape(16, 128, 1))
    brp = bias[1:].reshape(3, 16, 128)[:, PERM16]
    out["brep"] = np.ascontiguousarray(brp.transpose(0, 2, 1))  # (3,128,16)
    out["awct"] = np.ascontiguousarray(np.concatenate(
        [inputs["actor_w"], inputs["critic_w"]], 0).T.reshape(4, 128, 9)).astype(bf16)
    out["awcb"] = np.concatenate(
        [inputs["actor_b"], inputs["critic_b"]]).reshape(1, 9).astype(bf16)
    return out


def _prep_core(inputs, k):
    f32 = np.float32
    out = {}
    img = np.asarray(inputs["img"], f32).reshape(T, B, 3, 11, 9, 11, 9)
    imk = img[:, BL * k:BL * (k + 1)]                      # (64,4,3,11,9,11,9)
    im2 = np.ascontiguousarray(
        imk.transpose(2, 4, 6, 0, 1, 3, 5)).reshape(243, TBL * 121)
    im2 = np.concatenate([im2, np.zeros((13, TBL * 121), f32)], 0)
    out["im2col"] = im2.reshape(2, 128, TBL * 121).astype(bf16)
    lk = np.asarray(inputs["lang"], f32)[:, BL * k:BL * (k + 1)]   # (64,4,14)
    lm = np.ascontiguousarray(lk.transpose(2, 0, 1)).reshape(14, TBL)
    out["langm"] = np.concatenate([lm, np.zeros((2, TBL), f32)], 0).astype(bf16)
    out["donerow"] = np.ascontiguousarray(
        np.asarray(inputs["done"], f32)[:, BL * k:BL * (k + 1)]).reshape(1, TBL)
    act = np.asarray(inputs["action"]).reshape(T, B)[:, BL * k:BL * (k + 1)].reshape(TBL)
    ohm = np.zeros((TBL, NACT), f32)
    ohm[np.arange(TBL), act.astype(np.int64)] = 1.0
    out["oh"] = ohm.reshape(2, 128, NACT)
    eh0 = np.ascontiguousarray(
        np.asarray(inputs["enc_h0"], f32)[BL * k:BL * (k + 1)].T)   # (256,4)
    ec0 = np.ascontiguousarray(
        np.asarray(inputs["enc_c0"], f32)[BL * k:BL * (k + 1)].T)
    out["eh0"] = eh0.reshape(2, 128, BL).astype(bf16)
    out["ec0"] = ec0.reshape(2, 128, BL)
    mh0 = np.ascontiguousarray(
        np.asarray(inputs["mem_h0"], f32)[:, BL * k:BL * (k + 1)].transpose(0, 2, 1))
    mc0 = np.ascontiguousarray(
        np.asarray(inputs["mem_c0"], f32)[:, BL * k:BL * (k + 1)].transpose(0, 2, 1))
    out["mh0"] = mh0.reshape(MEM_L, 4, 128, BL).astype(bf16)
    out["mc0"] = mc0.reshape(MEM_L, 4, 128, BL)
    return out


def kernel(**inputs):
    from concourse import bass_utils

    if "nc" not in _cache:
        _cache["nc"] = _build_nc()
    nc = _cache["nc"]

    shared = _prep_shared({k: np.asarray(v) for k, v in inputs.items()
                           if k not in ("img", "lang", "done", "action",
                                        "enc_h0", "enc_c0", "mem_h0", "mem_c0")})
    in_maps = []
    for k in range(NCORES):
        m = dict(shared)
        m.update(_prep_core(inputs, k))
        in_maps.append(m)

    res = bass_utils.run_bass_kernel_spmd(nc, in_maps, core_ids=list(range(NCORES)),
                                          trace=bool(int(os.environ.get("KERNEL_TRACE", "0"))))
    out_full = np.zeros((T, B, 3), np.float32)
    for k in range(NCORES):
        out_full[:, BL * k:BL * (k + 1)] = res.results[k]["out"].reshape(T, BL, 3)
    if os.environ.get("KERNEL_RESULT_STASH"):
        _cache["last_res"] = res
    return out_full.reshape(T * B, 3)


# revision 11
# speedup vs baseline: 1.5585x; 1.0075x over previous
"""Trainium2 Bass kernel for nn_Agent (conv encoder + masked LSTM scans + heads).

Sharding: data-parallel over batch B=32 across 8 cores (B_local=4). The
sequential T=64 scans run locally per core.

Structure (single fused program per core):
  - conv encoder chunks interleaved with the lang-LSTM scan steps (fills PE
    gaps left by the lang recurrence's activation chain).
  - mem LSTM restructured as a chunk-wavefront: layer l processes T-chunk k
    after layer l-1 finished chunk k. The input-side gates (W_ih @ x) are
    batched per chunk (N=64 moving columns) instead of per step (N=4),
    removing ~40% of the PE instructions from the sequential loop. Up to 4
    (layer, chunk) blocks are interleaved step-by-step so each block's
    activation chain hides under the other blocks' matmuls.
  - recurrent/input weights stored fp8(e4m3) scaled by 1024 (keeps values
    out of fp8-denormal range); the 1/1024 is folded into the sigmoid/tanh
    activation `scale` and the precomputed gate biases.
  - gates reordered (i,f,o | g) so one sigmoid ACT covers i,f,o.
"""
import os
import sys
import numpy as np

for p in ("/opt/trn_rl_repo",):
    if p not in sys.path:
        sys.path.insert(0, p)

import ml_dtypes

bf16 = ml_dtypes.bfloat16
f8 = ml_dtypes.float8_e4m3
FP8_SCALE = 1024.0          # keep fp8-stored weights out of denormal range
INV_SCALE = 1.0 / FP8_SCALE

T, B = 64, 32
NCORES = 8
BL = B // NCORES            # 4 envs per core
TBL = T * BL                # 256 images per core
LANG_DIM, ENC_H, EMB_D, MEM_H, MEM_L, MEM_IN, NACT = 14, 256, 32, 512, 4, 288, 8

CHUNK = 16                  # conv pipeline: images per chunk
NCHUNK = TBL // CHUNK       # 16 conv chunks
TC = 16                     # scan chunk (timesteps)
NTC = T // TC               # 4 scan chunks
CB = TC * BL                # 64 moving columns per scan-chunk batch

# gate m-tile permutations: [i, f, o, g] blocks
PERM16 = [0, 1, 2, 3, 4, 5, 6, 7, 12, 13, 14, 15, 8, 9, 10, 11]
PERM8 = [0, 1, 2, 3, 6, 7, 4, 5]

_cache = {}


def _build_nc():
    import concourse.bacc as bacc
    import concourse.tile as tile
    from concourse import mybir

    dt = mybir.dt
    AF = mybir.ActivationFunctionType
    AX = mybir.AxisListType
    F32, BF16, F8 = dt.float32, dt.bfloat16, dt.float8e4

    nc = bacc.Bacc("TRN2", target_bir_lowering=False, debug=False,
                   enable_asserts=False, num_devices=NCORES)

    def din(name, shape, dty):
        return nc.dram_tensor(name, list(shape), dty, kind="ExternalInput")

    # ---------------- DRAM I/O ----------------
    d_im2col = din("im2col", (2, 128, TBL * 121), BF16)   # K-tiles x part x cols
    d_langm = din("langm", (16, TBL), BF16)
    d_done = din("donerow", (1, TBL), F32)
    d_oh = din("oh", (2, 128, NACT), F32)
    d_eh0 = din("eh0", (2, 128, BL), BF16)
    d_ec0 = din("ec0", (2, 128, BL), F32)
    d_mh0 = din("mh0", (MEM_L, 4, 128, BL), BF16)
    d_mc0 = din("mc0", (MEM_L, 4, 128, BL), F32)
    d_w1 = din("w1", (2, 128, 32), BF16)
    d_c1b = din("c1b", (32, 1), F32)
    d_w2 = din("w2", (3, 96, 32), BF16)
    d_c2b = din("c2b", (32, 1), F32)
    d_w3 = din("w3", (3, 96, 32), BF16)
    d_c3b = din("c3b", (32, 1), F32)
    d_fcwp = din("fcwp", (12, 128, 256), BF16)
    d_fcwl = din("fcwl", (32, 256), BF16)
    d_fcb = din("fcb", (2, 128, 1), F32)
    d_ewih = din("ewih", (16, 1024), F8)
    d_ewhh = din("ewhh", (2, 128, 1024), F8)
    d_ebias = din("ebias", (8, 128, 1), F32)
    d_embt = din("embt", (2, 128, 32), BF16)
    d_embb = din("embb", (32, 1), F32)
    d_w0t = din("w0t", (3, 128, 2048), F8)
    d_wrt = din("wrt", (12, 128, 2048), F8)
    d_wht = din("wht", (16, 128, 2048), F8)
    d_b0 = din("b0", (16, 128, 1), F32)
    d_brep = din("brep", (3, 128, 16), F32)
    d_awct = din("awct", (4, 128, 9), BF16)
    d_awcb = din("awcb", (1, 9), BF16)
    d_out = nc.dram_tensor("out", [TBL, 3], F32, kind="ExternalOutput")

    C1 = CHUNK * 121
    C2 = CHUNK * 81
    C3 = CHUNK * 49

    with tile.TileContext(nc) as tc:
        with (
            tc.tile_pool(name="persist", bufs=1) as P,
            tc.tile_pool(name="io", bufs=2) as IO,
            tc.tile_pool(name="work", bufs=4) as W,
            tc.tile_pool(name="xbuf", bufs=2) as XB,
            tc.tile_pool(name="cpool", bufs=3) as CP,
            tc.tile_pool(name="ps", bufs=1, space="PSUM") as PS,
            tc.tile_pool(name="psc", bufs=2, space="PSUM") as PSC,
            tc.tile_pool(name="psh", bufs=3, space="PSUM") as PSH,
            tc.tile_pool(name="psb", bufs=2, space="PSUM") as PSB,
        ):
            # ---------- persistent weights / tables ----------
            def ld(dram_ap, shape, dty, name):
                t = P.tile(shape, dty, tag=name)
                nc.sync.dma_start(t[:], dram_ap)
                return t

            def ldm(dram_t, tile_t, nblk, blkw):
                nc.sync.dma_start(
                    tile_t[:].rearrange("p (i c) -> p i c", i=nblk),
                    dram_t[:].rearrange("i p c -> p i c"))

            # lang encoder weights first (needed immediately)
            langm = ld(d_langm[:], [16, TBL], BF16, "langm")
            ewih = ld(d_ewih[:], [16, 1024], F8, "ewih")
            ewhh = P.tile([128, 2048], F8, tag="ewhh")
            nc.sync.dma_start(
                ewhh[:].rearrange("p (i c) -> p i c", i=2),
                d_ewhh[:].rearrange("i p c -> p i c"))
            ebias = P.tile([128, 8], F32, tag="ebias")
            ldm(d_ebias, ebias, 8, 1)
            drow = P.tile([1, TBL], F32, tag="drow")
            nc.sync.dma_start(drow[:], d_done[:])

            # conv weights
            w1 = P.tile([128, 64], BF16, tag="w1")
            ldm(d_w1, w1, 2, 32)
            c1b = ld(d_c1b[:], [32, 1], F32, "c1b")
            w2 = P.tile([96, 96], BF16, tag="w2")
            ldm(d_w2, w2, 3, 32)
            c2b = ld(d_c2b[:], [32, 1], F32, "c2b")
            w3 = P.tile([96, 96], BF16, tag="w3")
            ldm(d_w3, w3, 3, 32)
            c3b = ld(d_c3b[:], [32, 1], F32, "c3b")
            fcw = P.tile([128, 12 * 256], BF16, tag="fcw")
            ldm(d_fcwp, fcw, 12, 256)
            fcwl = ld(d_fcwl[:], [32, 256], BF16, "fcwl")
            fcb = P.tile([128, 2], F32, tag="fcb")
            ldm(d_fcb, fcb, 2, 1)
            embt = P.tile([128, 64], BF16, tag="embt")
            ldm(d_embt, embt, 2, 32)
            embb = ld(d_embb[:], [32, 1], F32, "embb")

            # mem LSTM weights: issued on the scalar hwdge ring so they do
            # not queue ahead of the im2col chunk stream on the sync ring.
            w0 = P.tile([128, 3 * 2048], F8, tag="w0")
            ldm(d_w0t, w0, 3, 2048)
            wr = P.tile([128, 12 * 2048], F8, tag="wr")
            wh = P.tile([128, 16 * 2048], F8, tag="wh")
            b0 = P.tile([128, 16], F32, tag="b0")
            ldm(d_b0, b0, 16, 1)
            brep = P.tile([128, 3 * 16], F32, tag="brep")
            ldm(d_brep, brep, 3, 16)
            awct = P.tile([128, 36], BF16, tag="awct")
            ldm(d_awct, awct, 4, 9)
            awcb = ld(d_awcb[:], [1, 9], BF16, "awcb")
            oh = P.tile([128, 16], F32, tag="oh")
            ldm(d_oh, oh, 2, 8)
            ones1 = P.tile([1, 128], BF16, tag="ones1")
            nc.gpsimd.memset(ones1[:], 1.0)

            # ---------- state sequences ----------
            # langHs slot t holds lang-h_{t-1}; slot 0 = initial state.
            langHs = P.tile([128, (T + 1) * 8], BF16, tag="langHs")
            lhs_v = langHs[:].rearrange("p (t x) -> p t x", x=8)
            lhs_v4 = langHs[:].rearrange("p (t k b) -> p t k b", k=2, b=BL)
            nc.sync.dma_start(lhs_v4[:, 0],
                              d_eh0[:].rearrange("i p c -> p i c"))
            ec0 = CP.tile([128, 8], F32, tag="ec")
            nc.sync.dma_start(ec0[:].rearrange("p (i c) -> p i c", i=2),
                              d_ec0[:].rearrange("i p c -> p i c"))
            # hSeq[l] slot t holds mem-h_{l,t-1}
            hSeq = []
            hs_v = []
            hs_v4 = []
            for l in range(MEM_L):
                hs = P.tile([128, (T + 1) * 16], BF16, tag=f"hseq{l}")
                hSeq.append(hs)
                hs_v.append(hs[:].rearrange("p (t x) -> p t x", x=16))
                hs_v4.append(hs[:].rearrange("p (t k b) -> p t k b", k=4, b=BL))
                nc.sync.dma_start(hs_v4[l][:, 0],
                                  d_mh0[l].rearrange("i p c -> p i c"))
            mc = []
            for l in range(MEM_L):
                c_ = CP.tile([128, 16], F32, tag=f"mc{l}")
                nc.sync.dma_start(c_[:].rearrange("p (i c) -> p i c", i=4),
                                  d_mc0[l].rearrange("i p c -> p i c"))
                mc.append(c_)

            # done masks: (1-done) broadcast to 128 partitions via ones-matmul,
            # then replicated per hidden-tile.
            drow_i = P.tile([1, TBL], BF16, tag="drow_i")
            nc.scalar.activation(drow_i[:], drow[:], AF.Copy, scale=-1.0, bias=1.0)
            dbc_ps = PS.tile([128, TBL], F32, tag="pre")
            nc.tensor.matmul(dbc_ps[:], ones1[:, :], drow_i[:, :], start=True, stop=True)
            dbc = P.tile([128, TBL], BF16, tag="dbc")
            nc.scalar.activation(dbc[:], dbc_ps[:], AF.Copy)
            dbc_v = dbc[:].rearrange("p (t b) -> p t b", b=BL)
            donem_l = P.tile([128, 8 * T], BF16, tag="donem_l")
            dl_v = donem_l[:].rearrange("p (t x) -> p t x", x=8)
            for j in range(2):
                nc.vector.tensor_copy(dl_v[:, :, 4 * j:4 * j + 4], dbc_v)
            donem_m = P.tile([128, 16 * T], BF16, tag="donem_m")
            dm_v = donem_m[:].rearrange("p (t x) -> p t x", x=16)
            for j in range(4):
                nc.vector.tensor_copy(dm_v[:, :, 4 * j:4 * j + 4], dbc_v)

            # persistent activations
            Zl = P.tile([128, 8 * TBL], F32, tag="Zl")
            imgh = P.tile([128, 2 * TBL], BF16, tag="imgh")
            lange = P.tile([32, TBL], BF16, tag="lange")
            memH = P.tile([128, 4 * TBL], BF16, tag="memH")

            # ---------- lang LSTM: precompute input gates (scaled space) ----
            for m in range(8):
                zp = PS.tile([128, TBL], F32, tag="pre")
                nc.tensor.matmul(zp[:], ewih[:, 128 * m:128 * m + 128],
                                 langm[:], start=True, stop=True)
                nc.scalar.activation(Zl[:, TBL * m:TBL * (m + 1)], zp[:],
                                     AF.Identity, bias=ebias[:, m:m + 1])
            Zl_v = Zl[:].rearrange("p (m n) -> p m n", n=TBL)

            # ---------- lang scan step ----------
            ec_state = [ec0]

            def lang_pre(t):
                ec = ec_state[0]
                hm = W.tile([128, 8], BF16, tag="ehm")
                cm = W.tile([128, 8], F32, tag="ecm")
                dsl = donem_l[:, 8 * t:8 * t + 8]
                nc.vector.tensor_mul(hm[:], lhs_v[:, t], dsl)
                nc.vector.tensor_mul(cm[:], ec[:], dsl)
                return hm, cm

            def lang_mm(t, lg, hm):
                for m in range(8):
                    for kt in range(2):
                        nc.tensor.matmul(lg[:, 4 * m:4 * m + 4],
                                         ewhh[:, 1024 * kt + 128 * m:1024 * kt + 128 * m + 128],
                                         hm[:, 4 * kt:4 * kt + 4],
                                         start=(kt == 0), stop=(kt == 1))

            def lang_post(t, lg, cm):
                gl = W.tile([128, 32], F32, tag="gl")
                nc.vector.tensor_add(gl[:].rearrange("p (m n) -> p m n", n=4),
                                     lg.rearrange("p (m n) -> p m n", n=4),
                                     Zl_v[:, :, 4 * t:4 * t + 4])
                sif = W.tile([128, 24], F32, tag="esif")
                tg = W.tile([128, 8], F32, tag="etg")
                nc.scalar.activation(sif[:], gl[:, 0:24], AF.Sigmoid, scale=INV_SCALE)
                nc.scalar.activation(tg[:], gl[:, 24:32], AF.Tanh, scale=INV_SCALE)
                t1 = W.tile([128, 8], F32, tag="et1")
                t2 = W.tile([128, 8], F32, tag="et2")
                nc.vector.tensor_mul(t1[:], sif[:, 8:16], cm[:])
                nc.vector.tensor_mul(t2[:], sif[:, 0:8], tg[:])
                ec = CP.tile([128, 8], F32, tag="ec")
                nc.vector.tensor_add(ec[:], t1[:], t2[:])
                ec_state[0] = ec
                th = W.tile([128, 8], F32, tag="eth")
                nc.scalar.activation(th[:], ec[:], AF.Tanh)
                nc.vector.tensor_mul(lhs_v[:, t + 1], sif[:, 16:24], th[:])

            def lang_step(t, lg):
                hm, cm = lang_pre(t)
                lang_mm(t, lg, hm)
                lang_post(t, lg, cm)

            # ---------- conv chunk (stages separated by hook calls) --------
            def conv_chunk(ch, hook):
                im = IO.tile([128, 2 * C1], BF16, tag="im2")
                im_v = im[:].rearrange("p (i c) -> p i c", i=2)
                nc.sync.dma_start(
                    im_v, d_im2col[:, :, C1 * ch:C1 * (ch + 1)].rearrange(
                        "i p c -> p i c"))
                ca = im_v[:, 0]
                cb = im_v[:, 1]
                x13 = IO.tile([96, C1 + 24], BF16, tag="x13")
                for ns in range(4):  # 4 imgs per matmul: N=484
                    c1p = PSC.tile([32, 484], F32, tag="cv")
                    nc.tensor.matmul(c1p[:], w1[:, 0:32],
                                     ca[:, 484 * ns:484 * (ns + 1)], start=True, stop=False)
                    nc.tensor.matmul(c1p[:], w1[:, 32:64],
                                     cb[:, 484 * ns:484 * (ns + 1)], start=False, stop=True)
                    nc.scalar.activation(x13[0:32, 484 * ns:484 * (ns + 1)], c1p[:],
                                         AF.Relu, bias=c1b[:])
                hook(0)
                x13v = x13[0:32, 0:C1].rearrange("p (g c) -> p g c", c=121)
                for d, off in ((1, 11), (2, 22)):
                    nc.vector.tensor_copy(
                        x13[32 * d:32 * d + 32, 0:C1].rearrange(
                            "p (g c) -> p g c", c=121)[:, :, 0:121 - off],
                        x13v[:, :, off:121])
                x23 = IO.tile([96, C2 + 18], BF16, tag="x23")
                x13w = x13[:, 0:C1].rearrange("p (g a b) -> p g a b", a=11, b=11)
                for ns in range(4):  # 4 imgs per matmul: N=324
                    c2p = PSC.tile([32, 4, 9, 9], F32, tag="cv")
                    for j in range(3):
                        nc.tensor.matmul(c2p[:], w2[:, 32 * j:32 * j + 32],
                                         x13w[:, 4 * ns:4 * ns + 4, 0:9, j:j + 9],
                                         start=(j == 0), stop=(j == 2))
                    nc.scalar.activation(
                        x23[0:32, 324 * ns:324 * (ns + 1)],
                        c2p[:].rearrange("p g a b -> p (g a b)"), AF.Relu, bias=c2b[:])
                hook(1)
                x23v = x23[0:32, 0:C2].rearrange("p (g c) -> p g c", c=81)
                for d, off in ((1, 9), (2, 18)):
                    nc.vector.tensor_copy(
                        x23[32 * d:32 * d + 32, 0:C2].rearrange(
                            "p (g c) -> p g c", c=81)[:, :, 0:81 - off],
                        x23v[:, :, off:81])
                x34 = IO.tile([128, C3 + 3], BF16, tag="x34")
                x23w = x23[:, 0:C2].rearrange("p (g a b) -> p g a b", a=9, b=9)
                for ns in range(4):  # 4 imgs per matmul: N=196
                    c3p = PSC.tile([32, 4, 7, 7], F32, tag="cv")
                    for j in range(3):
                        nc.tensor.matmul(c3p[:], w3[:, 32 * j:32 * j + 32],
                                         x23w[:, 4 * ns:4 * ns + 4, 0:7, j:j + 7],
                                         start=(j == 0), stop=(j == 2))
                    nc.scalar.activation(
                        x34[0:32, 196 * ns:196 * (ns + 1)],
                        c3p[:].rearrange("p g a b -> p (g a b)"), AF.Relu, bias=c3b[:])
                hook(2)
                x34v = x34[0:32, 0:C3].rearrange("p (g c) -> p g c", c=49)
                for d in (1, 2, 3):
                    nc.vector.tensor_copy(
                        x34[32 * d:32 * d + 32, 0:C3].rearrange(
                            "p (g c) -> p g c", c=49)[:, :, 0:49 - d],
                        x34v[:, :, d:49])
                x34w = x34[:, 0:C3].rearrange("p (g c) -> p g c", c=49)
                for mt in range(2):
                    fp = PSC.tile([128, CHUNK], F32, tag="cv")
                    for q in range(12):
                        nc.tensor.matmul(fp[:],
                                         fcw[:, 256 * q + 128 * mt:256 * q + 128 * mt + 128],
                                         x34w[:, :, 4 * q:4 * q + 1].opt(),
                                         start=(q == 0), stop=False)
                    nc.tensor.matmul(fp[:], fcwl[:, 128 * mt:128 * mt + 128],
                                     x34w[0:32, :, 48:49].opt(), start=False, stop=True)
                    nc.scalar.activation(
                        imgh[:, TBL * mt + CHUNK * ch:TBL * mt + CHUNK * (ch + 1)],
                        fp[:], AF.Relu, bias=fcb[:, mt:mt + 1])
                hook(3)

            # ---------- lang embedding per scan-chunk ----------
            def emb_chunk(k):
                ep = PS.tile([32, CB], F32, tag="pre")
                for kt in range(2):
                    nc.tensor.matmul(ep[:], embt[:, 32 * kt:32 * kt + 32],
                                     lhs_v4[:, 1 + TC * k:1 + TC * (k + 1), kt],
                                     start=(kt == 0), stop=(kt == 1))
                nc.scalar.activation(lange[:, CB * k:CB * (k + 1)], ep[:],
                                     AF.Relu, bias=embb[:])

            # ---------- mem scan: per-chunk input-gate batch ----------
            wrb = [None] * MEM_L

            def mem_batch(l, k):
                wb = XB.tile([128, 16 * CB], F32, tag=f"wrb{l}")
                wb_v = wb[:].rearrange("p (m c) -> p m c", c=CB)
                for half in range(2):
                    bp = PSB.tile([128, 8 * CB], F32, tag="bp")
                    bp_v = bp[:].rearrange("p (m c) -> p m c", c=CB)
                    for mm in range(8):
                        m = 8 * half + mm
                        if l == 0:
                            for kt in range(2):
                                nc.tensor.matmul(
                                    bp_v[:, mm],
                                    w0[:, 2048 * kt + 128 * m:2048 * kt + 128 * m + 128],
                                    imgh[:, TBL * kt + CB * k:TBL * kt + CB * (k + 1)],
                                    start=(kt == 0), stop=False)
                            nc.tensor.matmul(
                                bp_v[:, mm],
                                w0[0:32, 4096 + 128 * m:4096 + 128 * m + 128],
                                lange[:, CB * k:CB * (k + 1)],
                                start=False, stop=True)
                        else:
                            base = (l - 1) * 4 * 2048
                            for kt in range(4):
                                nc.tensor.matmul(
                                    bp_v[:, mm],
                                    wr[:, base + 2048 * kt + 128 * m:base + 2048 * kt + 128 * m + 128],
                                    hs_v4[l - 1][:, 1 + TC * k:1 + TC * (k + 1), kt],
                                    start=(kt == 0), stop=(kt == 3))
                        bias = b0[:, m:m + 1] if l == 0 else \
                            brep[:, 16 * (l - 1) + m:16 * (l - 1) + m + 1]
                        nc.scalar.activation(wb_v[:, m], bp_v[:, mm],
                                             AF.Identity, bias=bias)
                wrb[l] = wb[:].rearrange("p (m tl b) -> p m tl b", m=16, b=BL)

            # ---------- mem scan step ----------
            def mem_pre(l, t):
                dsl = donem_m[:, 16 * t:16 * t + 16]
                hm = W.tile([128, 16], BF16, tag=f"hm{l}")
                cm = W.tile([128, 16], F32, tag=f"cm{l}")
                nc.vector.tensor_mul(hm[:], hs_v[l][:, t], dsl)
                nc.vector.tensor_mul(cm[:], mc[l][:], dsl)
                return hm, cm

            def mem_mm(l, t, gp, hm):
                base = l * 4 * 2048
                for m in range(16):
                    for kt in range(4):
                        nc.tensor.matmul(
                            gp[:, 4 * m:4 * m + 4],
                            wh[:, base + 2048 * kt + 128 * m:base + 2048 * kt + 128 * m + 128],
                            hm[:, 4 * kt:4 * kt + 4],
                            start=(kt == 0), stop=(kt == 3))

            def mem_post(l, t, gp, cm):
                gs = W.tile([128, 64], F32, tag=f"gs{l}")
                nc.vector.tensor_add(gs[:].rearrange("p (m n) -> p m n", n=4),
                                     gp.rearrange("p (m n) -> p m n", n=4),
                                     wrb[l][:, :, t % TC])
                sif = W.tile([128, 48], F32, tag=f"msif{l}")
                tg = W.tile([128, 16], F32, tag=f"mtg{l}")
                nc.scalar.activation(sif[:], gs[:, 0:48], AF.Sigmoid, scale=INV_SCALE)
                nc.scalar.activation(tg[:], gs[:, 48:64], AF.Tanh, scale=INV_SCALE)
                t1 = W.tile([128, 16], F32, tag=f"mt1{l}")
                t2 = W.tile([128, 16], F32, tag=f"mt2{l}")
                nc.vector.tensor_mul(t1[:], sif[:, 16:32], cm[:])
                nc.vector.tensor_mul(t2[:], sif[:, 0:16], tg[:])
                c_ = CP.tile([128, 16], F32, tag=f"mc{l}")
                nc.vector.tensor_add(c_[:], t1[:], t2[:])
                mc[l] = c_
                th = W.tile([128, 16], F32, tag=f"mth{l}")
                nc.scalar.activation(th[:], c_[:], AF.Tanh)
                nc.vector.tensor_mul(hs_v[l][:, t + 1], sif[:, 32:48], th[:])

            # copy hSeq[3] chunk into kt-major memH layout for the heads
            def memh_chunk(k):
                mH = memH[:].rearrange("p (kt t b) -> p kt t b", kt=4, b=BL)
                for kt in range(4):
                    nc.vector.tensor_copy(
                        mH[:, kt, TC * k:TC * (k + 1)],
                        hs_v4[3][:, 1 + TC * k:1 + TC * (k + 1), kt])

            # ---------- heads ----------
            def heads(it):
                hd = PS.tile([128, 9], F32, tag="pre")
                for kt in range(4):
                    nc.tensor.matmul(hd[:],
                                     memH[:, TBL * kt + 128 * it:TBL * kt + 128 * it + 128],
                                     awct[:, 9 * kt:9 * kt + 9],
                                     start=(kt == 0), stop=False)
                nc.tensor.matmul(hd[:], ones1[:, :], awcb[:, :], start=False, stop=True)
                mx = W.tile([128, 1], F32, tag="hmx")
                nc.vector.reduce_max(mx[:], hd[:, 0:8], axis=AX.X)
                xm = W.tile([128, 8], F32, tag="hxm")
                nc.vector.tensor_scalar_sub(xm[:], hd[:, 0:8], mx[:])
                ex = W.tile([128, 8], F32, tag="hex")
                se = W.tile([128, 1], F32, tag="hse")
                nc.scalar.activation(ex[:], xm[:], AF.Exp, accum_out=se[:])
                lnv = W.tile([128, 1], F32, tag="hln")
                nc.scalar.activation(lnv[:], se[:], AF.Ln)
                logp = W.tile([128, 8], F32, tag="hlp")
                nc.vector.tensor_scalar_sub(logp[:], xm[:], lnv[:])
                lp1 = W.tile([128, 8], F32, tag="hlp1")
                nc.vector.tensor_mul(lp1[:], logp[:], oh[:, 8 * it:8 * it + 8])
                lpa = W.tile([128, 1], F32, tag="hlpa")
                nc.vector.reduce_sum(lpa[:], lp1[:], axis=AX.X)
                t3 = W.tile([128, 8], F32, tag="ht3")
                nc.vector.tensor_mul(t3[:], ex[:], xm[:])
                sxm = W.tile([128, 1], F32, tag="hsxm")
                nc.vector.reduce_sum(sxm[:], t3[:], axis=AX.X)
                rse = W.tile([128, 1], F32, tag="hrse")
                nc.vector.reciprocal(rse[:], se[:])
                m1 = W.tile([128, 1], F32, tag="hm1")
                nc.vector.tensor_mul(m1[:], sxm[:], rse[:])
                ent = W.tile([128, 1], F32, tag="hent")
                nc.vector.tensor_sub(ent[:], lnv[:], m1[:])
                osb = W.tile([128, 3], F32, tag="osb")
                nc.vector.tensor_copy(osb[:, 0:1], lpa[:])
                nc.vector.tensor_copy(osb[:, 1:2], ent[:])
                nc.vector.tensor_copy(osb[:, 2:3], hd[:, 8:9])
                nc.sync.dma_start(d_out[128 * it:128 * (it + 1), :], osb[:])

            # ---------- unified slot schedule ----------
            # slot s: conv chunks 4s..4s+3 (s<4) + mem blocks (l, k=s-1-l)
            lang_t = [0]
            for s in range(NTC + MEM_L):
                active = [(l, s - 1 - l) for l in range(MEM_L)
                          if 0 <= s - 1 - l < NTC]
                for (l, k) in active:
                    mem_batch(l, k)

                def hook(q, hi):
                    hps = PSH.tile([128, 320], F32, tag="hps")
                    do_lang = lang_t[0] < T and s < NTC
                    if do_lang:
                        lt = lang_t[0]
                        lang_t[0] += 1
                        lhm, lcm = lang_pre(lt)
                    pres = [mem_pre(l, TC * k + 4 * q + hi)
                            for (l, k) in active]
                    if do_lang:
                        lang_mm(lt, hps[:, 256:288], lhm)
                    for bi, (l, k) in enumerate(active):
                        mem_mm(l, TC * k + 4 * q + hi,
                               hps[:, 64 * bi:64 * bi + 64], pres[bi][0])
                    if do_lang:
                        lang_post(lt, hps[:, 256:288], lcm)
                    for bi, (l, k) in enumerate(active):
                        mem_post(l, TC * k + 4 * q + hi,
                                 hps[:, 64 * bi:64 * bi + 64], pres[bi][1])

                for q in range(4):
                    if s < NTC:
                        conv_chunk(4 * s + q,
                                   lambda hi, q=q: hook(q, hi))
                    else:
                        for hi in range(4):
                            hook(q, hi)
                if s == 0:
                    # big recurrent weights: issued here so the slot-0
                    # im2col stream is not queued behind them
                    ldm(d_wht, wh, 16, 2048)
                if s == 1:
                    ldm(d_wrt, wr, 12, 2048)
                if s < NTC:
                    emb_chunk(s)
                for (l, k) in active:
                    if l == 3:
                        memh_chunk(k)
                        if k == 1:
                            heads(0)
                        if k == 3:
                            heads(1)

    nc.compile()
    return nc


def _prep_shared(inputs):
    f32 = np.float32

    def permg(w, nblk, perm):
        # permute gate blocks on the LAST axis
        s = w.shape
        v = w.reshape(s[:-1] + (nblk, s[-1] // nblk))
        return np.ascontiguousarray(v[..., perm, :]).reshape(s)

    out = {}
    w1p = np.concatenate([inputs["conv1_w"].reshape(16, 243).T.astype(f32),
                          np.zeros((13, 16), f32)], 0)          # (256,16)
    out["w1"] = np.concatenate([w1p, w1p], 1).reshape(2, 128, 32).astype(bf16)
    out["c1b"] = np.concatenate([inputs["conv1_b"]] * 2).reshape(32, 1).astype(f32)
    w2z = np.zeros((3, 96, 32), f32)
    for j in range(3):
        for d in range(3):
            w2z[j, 32 * d:32 * d + 16, :] = inputs["conv2_w"][:, :, d, j].T
    out["w2"] = w2z.astype(bf16)
    out["c2b"] = inputs["conv2_b"].reshape(32, 1).astype(f32)
    w3 = np.stack([inputs["conv3_w"][:, :, d, :] for d in range(3)])  # (3,32,32,3)
    out["w3"] = np.ascontiguousarray(
        w3.transpose(3, 0, 2, 1).reshape(3, 96, 32)).astype(bf16)
    out["c3b"] = inputs["conv3_b"].reshape(32, 1).astype(f32)
    F = inputs["fc_w"].reshape(256, 32, 49)
    fcwp = np.stack([
        np.ascontiguousarray(F[:, :, 4 * q:4 * q + 4].transpose(2, 1, 0)).reshape(128, 256)
        for q in range(12)])
    out["fcwp"] = fcwp.astype(bf16)
    out["fcwl"] = np.ascontiguousarray(F[:, :, 48].T).astype(bf16)
    out["fcb"] = inputs["fc_b"].reshape(2, 128, 1).astype(f32)
    ewih = permg(np.concatenate(
        [inputs["enc_Wih"].T.astype(f32), np.zeros((2, 1024), f32)], 0),
        8, PERM8)
    out["ewih"] = (ewih * FP8_SCALE).astype(f8)
    ewhh = permg(inputs["enc_Whh"].T.astype(f32), 8, PERM8)
    out["ewhh"] = (np.ascontiguousarray(
        ewhh.reshape(2, 128, 1024)) * FP8_SCALE).astype(f8)
    ebias = permg((inputs["enc_bih"] + inputs["enc_bhh"]).astype(f32), 8, PERM8)
    out["ebias"] = (ebias * FP8_SCALE).reshape(8, 128, 1).astype(f32)
    out["embt"] = np.ascontiguousarray(
        inputs["emb_w"].T.reshape(2, 128, 32)).astype(bf16)
    out["embb"] = inputs["emb_b"].reshape(32, 1).astype(f32)
    w0 = permg(np.concatenate([inputs["mem_Wih0"].T.astype(f32),
                               np.zeros((96, 2048), f32)], 0), 16, PERM16)
    out["w0t"] = (w0.reshape(3, 128, 2048) * FP8_SCALE).astype(f8)
    wrt = permg(np.ascontiguousarray(
        inputs["mem_WihR"].transpose(0, 2, 1)).astype(f32), 16, PERM16)
    out["wrt"] = (wrt.reshape(12, 128, 2048) * FP8_SCALE).astype(f8)
    wht = permg(np.ascontiguousarray(
        inputs["mem_Whh"].transpose(0, 2, 1)).astype(f32), 16, PERM16)
    out["wht"] = (wht.reshape(16, 128, 2048) * FP8_SCALE).astype(f8)
    bias = ((inputs["mem_bih"] + inputs["mem_bhh"]) * FP8_SCALE).astype(f32)
    b0p = bias[0].reshape(16, 128)[PERM16]
    out["b0"] = np.ascontiguousarray(b0p.reshape(16, 128, 1))
    brp = bias[1:].reshape(3, 16, 128)[:, PERM16]
    out["brep"] = np.ascontiguousarray(brp.transpose(0, 2, 1))  # (3,128,16)
    out["awct"] = np.ascontiguousarray(np.concatenate(
        [inputs["actor_w"], inputs["critic_w"]], 0).T.reshape(4, 128, 9)).astype(bf16)
    out["awcb"] = np.concatenate(
        [inputs["actor_b"], inputs["critic_b"]]).reshape(1, 9).astype(bf16)
    return out


def _prep_core(inputs, k):
    f32 = np.float32
    out = {}
    img = np.asarray(inputs["img"], f32).reshape(T, B, 3, 11, 9, 11, 9)
    imk = img[:, BL * k:BL * (k + 1)]                      # (64,4,3,11,9,11,9)
    im2 = np.ascontiguousarray(
        imk.transpose(2, 4, 6, 0, 1, 3, 5)).reshape(243, TBL * 121)
    im2 = np.concatenate([im2, np.zeros((13, TBL * 121), f32)], 0)
    out["im2col"] = im2.reshape(2, 128, TBL * 121).astype(bf16)
    lk = np.asarray(inputs["lang"], f32)[:, BL * k:BL * (k + 1)]   # (64,4,14)
    lm = np.ascontiguousarray(lk.transpose(2, 0, 1)).reshape(14, TBL)
    out["langm"] = np.concatenate([lm, np.zeros((2, TBL), f32)], 0).astype(bf16)
    out["donerow"] = np.ascontiguousarray(
        np.asarray(inputs["done"], f32)[:, BL * k:BL * (k + 1)]).reshape(1, TBL)
    act = np.asarray(inputs["action"]).reshape(T, B)[:, BL * k:BL * (k + 1)].reshape(TBL)
    ohm = np.zeros((TBL, NACT), f32)
    ohm[np.arange(TBL), act.astype(np.int64)] = 1.0
    out["oh"] = ohm.reshape(2, 128, NACT)
    eh0 = np.ascontiguousarray(
        np.asarray(inputs["enc_h0"], f32)[BL * k:BL * (k + 1)].T)   # (256,4)
    ec0 = np.ascontiguousarray(
        np.asarray(inputs["enc_c0"], f32)[BL * k:BL * (k + 1)].T)
    out["eh0"] = eh0.reshape(2, 128, BL).astype(bf16)
    out["ec0"] = ec0.reshape(2, 128, BL)
    mh0 = np.ascontiguousarray(
        np.asarray(inputs["mem_h0"], f32)[:, BL * k:BL * (k + 1)].transpose(0, 2, 1))
    mc0 = np.ascontiguousarray(
        np.asarray(inputs["mem_c0"], f32)[:, BL * k:BL * (k + 1)].transpose(0, 2, 1))
    out["mh0"] = mh0.reshape(MEM_L, 4, 128, BL).astype(bf16)
    out["mc0"] = mc0.reshape(MEM_L, 4, 128, BL)
    return out


def kernel(**inputs):
    from concourse import bass_utils

    if "nc" not in _cache:
        _cache["nc"] = _build_nc()
    nc = _cache["nc"]

    shared = _prep_shared({k: np.asarray(v) for k, v in inputs.items()
                           if k not in ("img", "lang", "done", "action",
                                        "enc_h0", "enc_c0", "mem_h0", "mem_c0")})
    in_maps = []
    for k in range(NCORES):
        m = dict(shared)
        m.update(_prep_core(inputs, k))
        in_maps.append(m)

    res = bass_utils.run_bass_kernel_spmd(nc, in_maps, core_ids=list(range(NCORES)),
                                          trace=bool(int(os.environ.get("KERNEL_TRACE", "0"))))
    out_full = np.zeros((T, B, 3), np.float32)
    for k in range(NCORES):
        out_full[:, BL * k:BL * (k + 1)] = res.results[k]["out"].reshape(T, BL, 3)
    if os.environ.get("KERNEL_RESULT_STASH"):
        _cache["last_res"] = res
    return out_full.reshape(T * B, 3)
